# revision 4
# baseline (speedup 1.0000x reference)
"""Trainium2 Bass kernel for a full MHA block (proj -> masked softmax attention
-> fc -> residual -> layernorm), data-parallel over batch across 8 NeuronCores.

Layout strategy (per core, one batch element):
  - Host pre-transposes weights (W.T) and activations (q.T/k.T/v.T) so every
    matmul contraction dim lands on SBUF partitions with zero on-chip
    transposes.
  - Scores are computed *transposed* (S.T[lk, q]) so that the attention
    probabilities are directly usable as the moving operand of the attn@v
    matmul (contraction over lk = partitions).
  - Softmax without max-subtraction: raw scores are bounded (|S|/16 < ~10),
    masked entries get -2^30 added pre-exp so exp underflows to exactly 0.
    Fully-masked rows then produce sum==0 -> recip(sum+1e-30) finite -> attn
    row exactly 0, matching the reference's NaN-fix.
  - Sum-of-exp over partitions via a ones-vector matmul on the PE; the
    reciprocal row is broadcast back to 128 partitions with a K=1 matmul.
"""

import sys

if "/opt/trn_rl_repo" not in sys.path:
    sys.path.insert(0, "/opt/trn_rl_repo")

from contextlib import ExitStack

import ml_dtypes
import numpy as np

import concourse.bass as bass
import concourse.tile as tile
from concourse import bacc, mybir
from concourse.bass_utils import run_bass_kernel_spmd

B, L, D, H = 8, 2048, 1024, 4
DK = D // H  # 256
LT = L // 128  # 16 l-tiles of 128
DT = D // 128  # 8 d-tiles of 128
NQ = 4  # attention q chunks
QW = L // NQ  # 512 q columns per chunk
EPS = 1e-5
INV_TEMP = 1.0 / 16.0  # 1/sqrt(DK)
MASK_NEG = -float(2**30)

f32 = mybir.dt.float32
bf16 = mybir.dt.bfloat16

AF = mybir.ActivationFunctionType

# set by kernel() after each run; test.py reads it for the HW time
LAST_RESULT = None


def build(use_bqk, use_bv, use_bfc, use_gamma, use_beta):
    nc = bacc.Bacc("TRN2", target_bir_lowering=False, debug=False, num_devices=B)

    # ---- per-core I/O (full tensors for one batch element) ----
    qT = nc.dram_tensor("qT", [D, L], f32, kind="ExternalInput")
    kT = nc.dram_tensor("kT", [D, L], f32, kind="ExternalInput")
    vT = nc.dram_tensor("vT", [D, L], f32, kind="ExternalInput")
    q_nat = nc.dram_tensor("q_nat", [L, D], f32, kind="ExternalInput")
    maskbT = nc.dram_tensor("maskbT", [L, L], bf16, kind="ExternalInput")
    WqT = nc.dram_tensor("WqT", [D, D], f32, kind="ExternalInput")
    WkT = nc.dram_tensor("WkT", [D, D], f32, kind="ExternalInput")
    WvT = nc.dram_tensor("WvT", [D, D], f32, kind="ExternalInput")
    WfcT = nc.dram_tensor("WfcT", [D, D], f32, kind="ExternalInput")
    if use_bqk:
        # bias for dout tile j lives in column j as a [128,1] per-partition vec
        bq_cols = nc.dram_tensor("bq_cols", [128, DT], f32, kind="ExternalInput")
        bk_cols = nc.dram_tensor("bk_cols", [128, DT], f32, kind="ExternalInput")
    if use_bv:
        bv_row = nc.dram_tensor("bv_row", [1, D], f32, kind="ExternalInput")
    if use_bfc:
        bfc_row = nc.dram_tensor("bfc_row", [1, D], f32, kind="ExternalInput")
    if use_gamma:
        gamma_row = nc.dram_tensor("gamma_row", [1, D], f32, kind="ExternalInput")
    if use_beta:
        beta_row = nc.dram_tensor("beta_row", [1, D], f32, kind="ExternalInput")
    out = nc.dram_tensor("out", [L, D], f32, kind="ExternalOutput")

    with TileCtx(nc) as tc, ExitStack() as top:
        dram = top.enter_context(tc.tile_pool(name="dram", bufs=1, space="DRAM"))
        qpT = dram.tile([D, L], f32, name="qpT")  # [dout, l]
        kpT = dram.tile([D, L], f32, name="kpT")  # [dout, l]
        vp = dram.tile([L, D], f32, name="vp")  # [l, dout]
        avTn = dram.tile([D, L], f32, name="avTn")  # [dconcat, q] normalized

        singles = top.enter_context(tc.tile_pool(name="singles", bufs=1))
        ones_col = singles.tile([128, 1], f32, name="ones_col")
        nc.vector.memset(ones_col, 1.0)
        ones_row = singles.tile([1, 128], f32, name="ones_row")
        nc.vector.memset(ones_row, 1.0)
        eps_t = singles.tile([128, 1], f32, name="eps_t")
        nc.vector.memset(eps_t, EPS)
        if use_bqk:
            bq_sb = singles.tile([128, DT], f32, name="bq_sb")
            nc.sync.dma_start(bq_sb, bq_cols[:, :])
            bk_sb = singles.tile([128, DT], f32, name="bk_sb")
            nc.sync.dma_start(bk_sb, bk_cols[:, :])
        if use_bv:
            bv_sb = singles.tile([1, D], f32, name="bv_sb")
            nc.sync.dma_start(bv_sb, bv_row[:, :])
        if use_bfc:
            bfc_sb = singles.tile([1, D], f32, name="bfc_sb")
            nc.sync.dma_start(bfc_sb, bfc_row[:, :])
        if use_gamma:
            gamma_bc = singles.tile([128, D], f32, name="gamma_bc")
            g = gamma_row[:, :]
            nc.gpsimd.dma_start(
                out=gamma_bc,
                in_=bass.AP(tensor=g.tensor, offset=g.offset, ap=[[0, 128], [1, D]]),
            )
        if use_beta:
            beta_bc = singles.tile([128, D], f32, name="beta_bc")
            bt = beta_row[:, :]
            nc.gpsimd.dma_start(
                out=beta_bc,
                in_=bass.AP(tensor=bt.tensor, offset=bt.offset, ap=[[0, 128], [1, D]]),
            )

        # =========== Phase P: projections ===========
        # q/k: out_pT[dout, l] = W @ x.T   (lhsT = W.T[din,dout], rhs = x.T[din,l])
        for name, WT_d, xT_d, outT, bias_sb in (
            ("q", WqT, qT, qpT, bq_sb if use_bqk else None),
            ("k", WkT, kT, kpT, bk_sb if use_bqk else None),
        ):
            with ExitStack() as ph:
                wp = ph.enter_context(tc.tile_pool(name=f"w_{name}", bufs=1))
                xp = ph.enter_context(tc.tile_pool(name=f"x_{name}", bufs=1))
                ev = ph.enter_context(tc.tile_pool(name=f"ev_{name}", bufs=3))
                ps = ph.enter_context(
                    tc.tile_pool(name=f"ps_{name}", bufs=2, space="PSUM")
                )
                w_sb = wp.tile([128, DT, D], f32, name=f"w_sb_{name}")
                nc.sync.dma_start(w_sb, WT_d.rearrange("(t p) n -> p t n", p=128))
                x_sb = xp.tile([128, DT, L], f32, name=f"x_sb_{name}")
                nc.sync.dma_start(x_sb, xT_d.rearrange("(t p) l -> p t l", p=128))
                for dout_t in range(DT):
                    pst = [
                        ps.tile([128, QW], f32, name=f"pp{name}{dout_t}_{i}", tag=f"pp{i}")
                        for i in range(NQ)
                    ]
                    for din in range(DT):
                        lhsT = w_sb[:, din, dout_t * 128 : (dout_t + 1) * 128]
                        for qc in range(NQ):
                            nc.tensor.matmul(
                                pst[qc],
                                lhsT,
                                x_sb[:, din, qc * QW : (qc + 1) * QW],
                                start=(din == 0),
                                stop=(din == DT - 1),
                            )
                    evt = ev.tile([128, L], f32, name=f"evt_{name}", tag="evt")
                    for qc in range(NQ):
                        dst = evt[:, qc * QW : (qc + 1) * QW]
                        if bias_sb is not None:
                            nc.scalar.activation(
                                dst,
                                pst[qc],
                                AF.Identity,
                                bias=bias_sb[:, dout_t : dout_t + 1],
                            )
                        else:
                            nc.vector.tensor_copy(dst, pst[qc])
                    nc.sync.dma_start(
                        outT[dout_t * 128 : (dout_t + 1) * 128, :], evt
                    )

        # v: vp[l, dout] = x.T.T @ W.T   (lhsT = v.T[din, l], rhs = W.T[din, dout])
        with ExitStack() as ph:
            wp = ph.enter_context(tc.tile_pool(name="w_v", bufs=1))
            xp = ph.enter_context(tc.tile_pool(name="x_v", bufs=1))
            ev = ph.enter_context(tc.tile_pool(name="ev_v", bufs=3))
            ps = ph.enter_context(tc.tile_pool(name="ps_v", bufs=2, space="PSUM"))
            w_sb = wp.tile([128, DT, D], f32, name="w_sb_v")
            nc.sync.dma_start(w_sb, WvT.rearrange("(t p) n -> p t n", p=128))
            x_sb = xp.tile([128, DT, L], f32, name="x_sb_v")
            nc.sync.dma_start(x_sb, vT.rearrange("(t p) l -> p t l", p=128))
            for l_t in range(LT):
                pst = [
                    ps.tile([128, 512], f32, name=f"ppv{l_t}_{i}", tag=f"ppv{i}")
                    for i in range(2)
                ]
                for din in range(DT):
                    lhsT = x_sb[:, din, l_t * 128 : (l_t + 1) * 128]
                    for dc in range(2):
                        nc.tensor.matmul(
                            pst[dc],
                            lhsT,
                            w_sb[:, din, dc * 512 : (dc + 1) * 512],
                            start=(din == 0),
                            stop=(din == DT - 1 and not use_bv),
                        )
                if use_bv:
                    for dc in range(2):
                        nc.tensor.matmul(
                            pst[dc],
                            ones_row,
                            bv_sb[:, dc * 512 : (dc + 1) * 512],
                            start=False,
                            stop=True,
                        )
                evt = ev.tile([128, D], f32, name="evt_v", tag="evtv")
                for dc in range(2):
                    nc.vector.tensor_copy(evt[:, dc * 512 : (dc + 1) * 512], pst[dc])
                nc.sync.dma_start(vp[l_t * 128 : (l_t + 1) * 128, :], evt)

        # =========== Phase A: attention per head ===========
        with ExitStack() as ph:
            mp = ph.enter_context(tc.tile_pool(name="maskp", bufs=1))
            mask_sb = mp.tile([128, LT, L], bf16, name="mask_sb")
            nc.sync.dma_start(mask_sb, maskbT.rearrange("(t p) q -> p t q", p=128))

            hp = ph.enter_context(tc.tile_pool(name="headp", bufs=2))
            wk = ph.enter_context(tc.tile_pool(name="attn_work", bufs=3))
            ps = ph.enter_context(tc.tile_pool(name="attn_ps", bufs=1, space="PSUM"))

            for h in range(H):
                r0 = h * DK
                kp_sb = hp.tile([128, 2, L], f32, name=f"kp_sb{h}", tag="kp")
                nc.sync.dma_start(
                    kp_sb, kpT[r0 : r0 + DK, :].rearrange("(c p) q -> p c q", p=128)
                )
                qp_sb = hp.tile([128, 2, L], f32, name=f"qp_sb{h}", tag="qp")
                nc.sync.dma_start(
                    qp_sb, qpT[r0 : r0 + DK, :].rearrange("(c p) q -> p c q", p=128)
                )
                vp_sb = hp.tile([128, LT, DK], f32, name=f"vp_sb{h}", tag="vph")
                nc.sync.dma_start(
                    vp_sb,
                    vp[:, r0 : r0 + DK].rearrange("(t p) d -> p t d", p=128),
                )
                for qc in range(NQ):
                    qs = slice(qc * QW, (qc + 1) * QW)
                    av_ps = [
                        ps.tile([128, QW], f32, name=f"av{h}_{qc}_{i}", tag=f"av{i}")
                        for i in range(2)
                    ]
                    sum_ps = ps.tile([1, QW], f32, name=f"sum{h}_{qc}", tag="sum")
                    for lk in range(LT):
                        st_ps = ps.tile(
                            [128, QW], f32, name=f"st{h}_{qc}_{lk}", tag="st", bufs=2
                        )
                        nc.tensor.matmul(
                            st_ps,
                            kp_sb[:, 0, lk * 128 : (lk + 1) * 128],
                            qp_sb[:, 0, qs],
                            start=True,
                            stop=False,
                        )
                        nc.tensor.matmul(
                            st_ps,
                            kp_sb[:, 1, lk * 128 : (lk + 1) * 128],
                            qp_sb[:, 1, qs],
                            start=False,
                            stop=True,
                        )
                        stm = wk.tile([128, QW], f32, name=f"stm{h}{qc}{lk}", tag="stm")
                        nc.vector.tensor_add(stm, st_ps, mask_sb[:, lk, qs])
                        pt = wk.tile([128, QW], f32, name=f"pt{h}{qc}{lk}", tag="pt")
                        nc.scalar.activation(pt, stm, AF.Exp, scale=INV_TEMP)
                        nc.tensor.matmul(
                            av_ps[0],
                            vp_sb[:, lk, 0:128],
                            pt,
                            start=(lk == 0),
                            stop=(lk == LT - 1),
                        )
                        nc.tensor.matmul(
                            av_ps[1],
                            vp_sb[:, lk, 128:256],
                            pt,
                            start=(lk == 0),
                            stop=(lk == LT - 1),
                        )
                        nc.tensor.matmul(
                            sum_ps,
                            ones_col,
                            pt,
                            start=(lk == 0),
                            stop=(lk == LT - 1),
                        )
                    srow = wk.tile([1, QW], f32, name=f"srow{h}{qc}", tag="srow")
                    nc.vector.tensor_scalar_add(srow, sum_ps, 1e-30)
                    nc.vector.reciprocal(srow, srow)
                    rb_ps = ps.tile([128, QW], f32, name=f"rb{h}_{qc}", tag="rb")
                    nc.tensor.matmul(rb_ps, ones_row, srow, start=True, stop=True)
                    rb_sb = wk.tile([128, QW], f32, name=f"rbs{h}{qc}", tag="rbs")
                    nc.scalar.copy(rb_sb, rb_ps)
                    for half in range(2):
                        avn = wk.tile(
                            [128, QW], f32, name=f"avn{h}{qc}{half}", tag=f"avn{half}"
                        )
                        nc.vector.tensor_mul(avn, av_ps[half], rb_sb)
                        rr = r0 + half * 128
                        nc.sync.dma_start(avTn[rr : rr + 128, qs], avn)

        # =========== Phase F: fc + residual + layernorm ===========
        with ExitStack() as ph:
            wp = ph.enter_context(tc.tile_pool(name="w_fc", bufs=1))
            ap_ = ph.enter_context(tc.tile_pool(name="avt", bufs=1))
            wk = ph.enter_context(tc.tile_pool(name="ln_work", bufs=3))
            ps = ph.enter_context(tc.tile_pool(name="fc_ps", bufs=2, space="PSUM"))
            w_sb = wp.tile([128, DT, D], f32, name="w_sb_fc")
            nc.sync.dma_start(w_sb, WfcT.rearrange("(t p) n -> p t n", p=128))
            av_sb = ap_.tile([128, DT, L], f32, name="av_sb")
            nc.sync.dma_start(av_sb, avTn.rearrange("(t p) q -> p t q", p=128))

            sd = nc.vector.BN_STATS_DIM
            for q_t in range(LT):
                qsl = slice(q_t * 128, (q_t + 1) * 128)
                resid = wk.tile([128, D], f32, name=f"res{q_t}", tag="resid")
                nc.sync.dma_start(resid, q_nat[qsl, :])
                fc_ps = [
                    ps.tile([128, 512], f32, name=f"fc{q_t}_{i}", tag=f"fc{i}")
                    for i in range(2)
                ]
                for dc in range(2):
                    for din in range(DT):
                        nc.tensor.matmul(
                            fc_ps[dc],
                            av_sb[:, din, q_t * 128 : (q_t + 1) * 128],
                            w_sb[:, din, dc * 512 : (dc + 1) * 512],
                            start=(din == 0),
                            stop=(din == DT - 1 and not use_bfc),
                        )
                    if use_bfc:
                        nc.tensor.matmul(
                            fc_ps[dc],
                            ones_row,
                            bfc_sb[:, dc * 512 : (dc + 1) * 512],
                            start=False,
                            stop=True,
                        )
                x = wk.tile([128, D], f32, name=f"x{q_t}", tag="x")
                for dc in range(2):
                    nc.vector.tensor_add(
                        x[:, dc * 512 : (dc + 1) * 512],
                        fc_ps[dc],
                        resid[:, dc * 512 : (dc + 1) * 512],
                    )
                stats = wk.tile([128, 2, sd], f32, name=f"stats{q_t}", tag="stats")
                nc.vector.bn_stats(stats[:, 0, :], x[:, 0:512])
                nc.vector.bn_stats(stats[:, 1, :], x[:, 512:1024])
                mv = wk.tile([128, nc.vector.BN_AGGR_DIM], f32, name=f"mv{q_t}", tag="mv")
                nc.vector.bn_aggr(mv, stats)
                rstd = wk.tile([128, 1], f32, name=f"rstd{q_t}", tag="rstd")
                nc.scalar.activation(rstd, mv[:, 1:2], AF.Sqrt, bias=eps_t)
                nc.vector.reciprocal(rstd, rstd)
                y = wk.tile([128, D], f32, name=f"y{q_t}", tag="y")
                nc.vector.tensor_scalar(
                    out=y,
                    in0=x,
                    scalar1=mv[:, 0:1],
                    scalar2=rstd,
                    op0=mybir.AluOpType.subtract,
                    op1=mybir.AluOpType.mult,
                )
                if use_gamma:
                    nc.vector.tensor_mul(y, y, gamma_bc)
                if use_beta:
                    nc.vector.tensor_add(y, y, beta_bc)
                nc.sync.dma_start(out[qsl, :], y)

    nc.compile()
    return nc


def TileCtx(nc):
    return tile.TileContext(nc)


_cache = {}


def _get_program(flags):
    key = tuple(sorted(flags.items()))
    if key not in _cache:
        _cache[key] = build(**flags)
    return _cache[key]


def kernel(**inputs):
    global LAST_RESULT
    q = np.asarray(inputs["q"], dtype=np.float32)
    k = np.asarray(inputs["k"], dtype=np.float32)
    v = np.asarray(inputs["v"], dtype=np.float32)
    mask = np.asarray(inputs["mask"])
    Wq = np.asarray(inputs["Wq"], dtype=np.float32)
    bq = np.asarray(inputs["bq"], dtype=np.float32)
    Wk = np.asarray(inputs["Wk"], dtype=np.float32)
    bk = np.asarray(inputs["bk"], dtype=np.float32)
    Wv = np.asarray(inputs["Wv"], dtype=np.float32)
    bv = np.asarray(inputs["bv"], dtype=np.float32)
    Wfc = np.asarray(inputs["Wfc"], dtype=np.float32)
    bfc = np.asarray(inputs["bfc"], dtype=np.float32)
    gamma = np.asarray(inputs["gamma"], dtype=np.float32)
    beta = np.asarray(inputs["beta"], dtype=np.float32)

    flags = dict(
        use_bqk=bool(np.any(bq) or np.any(bk)),
        use_bv=bool(np.any(bv)),
        use_bfc=bool(np.any(bfc)),
        use_gamma=bool(np.any(gamma != 1.0)),
        use_beta=bool(np.any(beta)),
    )
    nc = _get_program(flags)

    WqT = np.ascontiguousarray(Wq.T)
    WkT = np.ascontiguousarray(Wk.T)
    WvT = np.ascontiguousarray(Wv.T)
    WfcT = np.ascontiguousarray(Wfc.T)

    neg = np.array(MASK_NEG, dtype=ml_dtypes.bfloat16)
    zero = np.array(0.0, dtype=ml_dtypes.bfloat16)

    shared = dict(WqT=WqT, WkT=WkT, WvT=WvT, WfcT=WfcT)
    if flags["use_bqk"]:
        shared["bq_cols"] = np.ascontiguousarray(bq.reshape(DT, 128).T)
        shared["bk_cols"] = np.ascontiguousarray(bk.reshape(DT, 128).T)
    if flags["use_bv"]:
        shared["bv_row"] = bv.reshape(1, D)
    if flags["use_bfc"]:
        shared["bfc_row"] = bfc.reshape(1, D)
    if flags["use_gamma"]:
        shared["gamma_row"] = gamma.reshape(1, D)
    if flags["use_beta"]:
        shared["beta_row"] = beta.reshape(1, D)

    in_maps = []
    for b in range(B):
        m = dict(shared)
        m["qT"] = np.ascontiguousarray(q[b].T)
        m["kT"] = np.ascontiguousarray(k[b].T)
        m["vT"] = np.ascontiguousarray(v[b].T)
        m["q_nat"] = np.ascontiguousarray(q[b])
        m["maskbT"] = np.where(mask[b].T, neg, zero)
        in_maps.append(m)

    LAST_RESULT = run_bass_kernel_spmd(nc, in_maps, core_ids=list(range(B)))
    return np.stack([r["out"] for r in LAST_RESULT.results], axis=0)


# revision 5
# speedup vs baseline: 2.3503x; 2.3503x over previous
"""Trainium2 Bass kernel for a full MHA block (proj -> masked softmax attention
-> fc -> residual -> layernorm), data-parallel over batch across 8 NeuronCores.

Layout strategy (per core, one batch element):
  - Host pre-transposes weights (W.T) and activations (q.T/k.T/v.T) so every
    matmul contraction dim lands on SBUF partitions with zero on-chip
    transposes.
  - Scores are computed *transposed* (S.T[lk, q]) so that the attention
    probabilities are directly usable as the moving operand of the attn@v
    matmul (contraction over lk = partitions).
  - Softmax without max-subtraction: raw scores are bounded (|S|/16 < ~10),
    masked entries get -2^30 added pre-exp so exp underflows to exactly 0.
    Fully-masked rows then produce sum==0 -> recip(sum+1e-30) finite -> attn
    row exactly 0, matching the reference's NaN-fix.
  - Sum-of-exp over partitions via a ones-vector matmul on the PE; the
    reciprocal row is broadcast back to 128 partitions with a K=1 matmul.
"""

import sys

if "/opt/trn_rl_repo" not in sys.path:
    sys.path.insert(0, "/opt/trn_rl_repo")

from contextlib import ExitStack

import ml_dtypes
import numpy as np

import concourse.bass as bass
import concourse.tile as tile
from concourse import bacc, mybir
from concourse.bass_utils import run_bass_kernel_spmd

B, L, D, H = 8, 2048, 1024, 4
DK = D // H  # 256
LT = L // 128  # 16 l-tiles of 128
DT = D // 128  # 8 d-tiles of 128
NQ = 4  # attention q chunks
QW = L // NQ  # 512 q columns per chunk
EPS = 1e-5
INV_TEMP = 1.0 / 16.0  # 1/sqrt(DK)
MASK_NEG = -float(2**30)

f32 = mybir.dt.float32
bf16 = mybir.dt.bfloat16

AF = mybir.ActivationFunctionType

# set by kernel() after each run; test.py reads it for the HW time
LAST_RESULT = None


def build(use_bqk, use_bv, use_bfc, use_gamma, use_beta):
    nc = bacc.Bacc("TRN2", target_bir_lowering=False, debug=False, num_devices=B)

    # ---- per-core I/O (full tensors for one batch element) ----
    qT = nc.dram_tensor("qT", [D, L], bf16, kind="ExternalInput")
    kT = nc.dram_tensor("kT", [D, L], bf16, kind="ExternalInput")
    vT = nc.dram_tensor("vT", [D, L], bf16, kind="ExternalInput")
    q_nat = nc.dram_tensor("q_nat", [L, D], f32, kind="ExternalInput")
    maskbT = nc.dram_tensor("maskbT", [L, L], bf16, kind="ExternalInput")
    WqT = nc.dram_tensor("WqT", [D, D], bf16, kind="ExternalInput")
    WkT = nc.dram_tensor("WkT", [D, D], bf16, kind="ExternalInput")
    WvT = nc.dram_tensor("WvT", [D, D], bf16, kind="ExternalInput")
    WfcT = nc.dram_tensor("WfcT", [D, D], bf16, kind="ExternalInput")
    if use_bqk:
        # bias for dout tile j lives in column j as a [128,1] per-partition vec
        bq_cols = nc.dram_tensor("bq_cols", [128, DT], f32, kind="ExternalInput")
        bk_cols = nc.dram_tensor("bk_cols", [128, DT], f32, kind="ExternalInput")
    if use_bv:
        bv_row = nc.dram_tensor("bv_row", [1, D], f32, kind="ExternalInput")
    if use_bfc:
        bfc_row = nc.dram_tensor("bfc_row", [1, D], f32, kind="ExternalInput")
    if use_gamma:
        gamma_row = nc.dram_tensor("gamma_row", [1, D], f32, kind="ExternalInput")
    if use_beta:
        beta_row = nc.dram_tensor("beta_row", [1, D], f32, kind="ExternalInput")
    out = nc.dram_tensor("out", [L, D], f32, kind="ExternalOutput")

    with TileCtx(nc) as tc, ExitStack() as top:
        dram = top.enter_context(tc.tile_pool(name="dram", bufs=1, space="DRAM"))
        qpT = dram.tile([D, L], bf16, name="qpT")  # [dout, l]
        kpT = dram.tile([D, L], bf16, name="kpT")  # [dout, l]
        vp = dram.tile([L, D], bf16, name="vp")  # [l, dout]
        avTn = dram.tile([D, L], bf16, name="avTn")  # [dconcat, q] normalized

        singles = top.enter_context(tc.tile_pool(name="singles", bufs=1))
        ones_col = singles.tile([128, 1], bf16, name="ones_col")
        nc.vector.memset(ones_col, 1.0)
        ones_row = singles.tile([1, 128], f32, name="ones_row")
        nc.vector.memset(ones_row, 1.0)
        eps_t = singles.tile([128, 1], f32, name="eps_t")
        nc.vector.memset(eps_t, EPS)
        if use_bqk:
            bq_sb = singles.tile([128, DT], f32, name="bq_sb")
            nc.sync.dma_start(bq_sb, bq_cols[:, :])
            bk_sb = singles.tile([128, DT], f32, name="bk_sb")
            nc.sync.dma_start(bk_sb, bk_cols[:, :])
        if use_bv:
            bv_sb = singles.tile([1, D], f32, name="bv_sb")
            nc.sync.dma_start(bv_sb, bv_row[:, :])
        if use_bfc:
            bfc_sb = singles.tile([1, D], f32, name="bfc_sb")
            nc.sync.dma_start(bfc_sb, bfc_row[:, :])
        if use_gamma:
            gamma_bc = singles.tile([128, D], f32, name="gamma_bc")
            g = gamma_row[:, :]
            nc.gpsimd.dma_start(
                out=gamma_bc,
                in_=bass.AP(tensor=g.tensor, offset=g.offset, ap=[[0, 128], [1, D]]),
            )
        if use_beta:
            beta_bc = singles.tile([128, D], f32, name="beta_bc")
            bt = beta_row[:, :]
            nc.gpsimd.dma_start(
                out=beta_bc,
                in_=bass.AP(tensor=bt.tensor, offset=bt.offset, ap=[[0, 128], [1, D]]),
            )

        # =========== Phase P: projections ===========
        # q/k: out_pT[dout, l] = W @ x.T   (lhsT = W.T[din,dout], rhs = x.T[din,l])
        for name, WT_d, xT_d, outT, bias_sb in (
            ("q", WqT, qT, qpT, bq_sb if use_bqk else None),
            ("k", WkT, kT, kpT, bk_sb if use_bqk else None),
        ):
            with ExitStack() as ph:
                wp = ph.enter_context(tc.tile_pool(name=f"w_{name}", bufs=1))
                xp = ph.enter_context(tc.tile_pool(name=f"x_{name}", bufs=1))
                ev = ph.enter_context(tc.tile_pool(name=f"ev_{name}", bufs=3))
                ps = ph.enter_context(
                    tc.tile_pool(name=f"ps_{name}", bufs=2, space="PSUM")
                )
                w_sb = wp.tile([128, DT, D], bf16, name=f"w_sb_{name}")
                nc.sync.dma_start(w_sb, WT_d.rearrange("(t p) n -> p t n", p=128))
                x_sb = xp.tile([128, DT, L], bf16, name=f"x_sb_{name}")
                nc.sync.dma_start(x_sb, xT_d.rearrange("(t p) l -> p t l", p=128))
                for dout_t in range(DT):
                    pst = [
                        ps.tile([128, QW], f32, name=f"pp{name}{dout_t}_{i}", tag=f"pp{i}")
                        for i in range(NQ)
                    ]
                    for din in range(DT):
                        lhsT = w_sb[:, din, dout_t * 128 : (dout_t + 1) * 128]
                        for qc in range(NQ):
                            nc.tensor.matmul(
                                pst[qc],
                                lhsT,
                                x_sb[:, din, qc * QW : (qc + 1) * QW],
                                start=(din == 0),
                                stop=(din == DT - 1),
                            )
                    evt = ev.tile([128, L], bf16, name=f"evt_{name}", tag="evt")
                    for qc in range(NQ):
                        dst = evt[:, qc * QW : (qc + 1) * QW]
                        if bias_sb is not None:
                            nc.scalar.activation(
                                dst,
                                pst[qc],
                                AF.Identity,
                                bias=bias_sb[:, dout_t : dout_t + 1],
                            )
                        else:
                            nc.vector.tensor_copy(dst, pst[qc])
                    nc.sync.dma_start(
                        outT[dout_t * 128 : (dout_t + 1) * 128, :], evt
                    )

        # v: vp[l, dout] = x.T.T @ W.T   (lhsT = v.T[din, l], rhs = W.T[din, dout])
        with ExitStack() as ph:
            wp = ph.enter_context(tc.tile_pool(name="w_v", bufs=1))
            xp = ph.enter_context(tc.tile_pool(name="x_v", bufs=1))
            ev = ph.enter_context(tc.tile_pool(name="ev_v", bufs=3))
            ps = ph.enter_context(tc.tile_pool(name="ps_v", bufs=2, space="PSUM"))
            w_sb = wp.tile([128, DT, D], bf16, name="w_sb_v")
            nc.sync.dma_start(w_sb, WvT.rearrange("(t p) n -> p t n", p=128))
            x_sb = xp.tile([128, DT, L], bf16, name="x_sb_v")
            nc.sync.dma_start(x_sb, vT.rearrange("(t p) l -> p t l", p=128))
            for l_t in range(LT):
                pst = [
                    ps.tile([128, 512], f32, name=f"ppv{l_t}_{i}", tag=f"ppv{i}")
                    for i in range(2)
                ]
                for din in range(DT):
                    lhsT = x_sb[:, din, l_t * 128 : (l_t + 1) * 128]
                    for dc in range(2):
                        nc.tensor.matmul(
                            pst[dc],
                            lhsT,
                            w_sb[:, din, dc * 512 : (dc + 1) * 512],
                            start=(din == 0),
                            stop=(din == DT - 1 and not use_bv),
                        )
                if use_bv:
                    for dc in range(2):
                        nc.tensor.matmul(
                            pst[dc],
                            ones_row,
                            bv_sb[:, dc * 512 : (dc + 1) * 512],
                            start=False,
                            stop=True,
                        )
                evt = ev.tile([128, D], bf16, name="evt_v", tag="evtv")
                for dc in range(2):
                    nc.vector.tensor_copy(evt[:, dc * 512 : (dc + 1) * 512], pst[dc])
                nc.sync.dma_start(vp[l_t * 128 : (l_t + 1) * 128, :], evt)

        # =========== Phase A: attention per head ===========
        with ExitStack() as ph:
            mp = ph.enter_context(tc.tile_pool(name="maskp", bufs=1))
            mask_sb = mp.tile([128, LT, L], bf16, name="mask_sb")
            nc.sync.dma_start(mask_sb, maskbT.rearrange("(t p) q -> p t q", p=128))

            hp = ph.enter_context(tc.tile_pool(name="headp", bufs=2))
            wk = ph.enter_context(tc.tile_pool(name="attn_work", bufs=3))
            ps = ph.enter_context(tc.tile_pool(name="attn_ps", bufs=1, space="PSUM"))

            for h in range(H):
                r0 = h * DK
                kp_sb = hp.tile([128, 2, L], bf16, name=f"kp_sb{h}", tag="kp")
                nc.sync.dma_start(
                    kp_sb, kpT[r0 : r0 + DK, :].rearrange("(c p) q -> p c q", p=128)
                )
                qp_sb = hp.tile([128, 2, L], bf16, name=f"qp_sb{h}", tag="qp")
                nc.sync.dma_start(
                    qp_sb, qpT[r0 : r0 + DK, :].rearrange("(c p) q -> p c q", p=128)
                )
                vp_sb = hp.tile([128, LT, DK], bf16, name=f"vp_sb{h}", tag="vph")
                nc.sync.dma_start(
                    vp_sb,
                    vp[:, r0 : r0 + DK].rearrange("(t p) d -> p t d", p=128),
                )
                for qc in range(NQ):
                    qs = slice(qc * QW, (qc + 1) * QW)
                    av_ps = [
                        ps.tile([128, QW], f32, name=f"av{h}_{qc}_{i}", tag=f"av{i}")
                        for i in range(2)
                    ]
                    sum_ps = ps.tile([1, QW], f32, name=f"sum{h}_{qc}", tag="sum")
                    for lk in range(LT):
                        st_ps = ps.tile(
                            [128, QW], f32, name=f"st{h}_{qc}_{lk}", tag="st", bufs=2
                        )
                        nc.tensor.matmul(
                            st_ps,
                            kp_sb[:, 0, lk * 128 : (lk + 1) * 128],
                            qp_sb[:, 0, qs],
                            start=True,
                            stop=False,
                        )
                        nc.tensor.matmul(
                            st_ps,
                            kp_sb[:, 1, lk * 128 : (lk + 1) * 128],
                            qp_sb[:, 1, qs],
                            start=False,
                            stop=True,
                        )
                        stm = wk.tile([128, QW], f32, name=f"stm{h}{qc}{lk}", tag="stm")
                        nc.vector.tensor_add(stm, st_ps, mask_sb[:, lk, qs])
                        pt = wk.tile([128, QW], bf16, name=f"pt{h}{qc}{lk}", tag="pt")
                        nc.scalar.activation(pt, stm, AF.Exp, scale=INV_TEMP)
                        nc.tensor.matmul(
                            av_ps[0],
                            vp_sb[:, lk, 0:128],
                            pt,
                            start=(lk == 0),
                            stop=(lk == LT - 1),
                        )
                        nc.tensor.matmul(
                            av_ps[1],
                            vp_sb[:, lk, 128:256],
                            pt,
                            start=(lk == 0),
                            stop=(lk == LT - 1),
                        )
                        nc.tensor.matmul(
                            sum_ps,
                            ones_col,
                            pt,
                            start=(lk == 0),
                            stop=(lk == LT - 1),
                        )
                    srow = wk.tile([1, QW], f32, name=f"srow{h}{qc}", tag="srow")
                    nc.vector.tensor_scalar_add(srow, sum_ps, 1e-30)
                    nc.vector.reciprocal(srow, srow)
                    rb_ps = ps.tile([128, QW], f32, name=f"rb{h}_{qc}", tag="rb")
                    nc.tensor.matmul(rb_ps, ones_row, srow, start=True, stop=True)
                    rb_sb = wk.tile([128, QW], f32, name=f"rbs{h}{qc}", tag="rbs")
                    nc.scalar.copy(rb_sb, rb_ps)
                    for half in range(2):
                        avn = wk.tile(
                            [128, QW], bf16, name=f"avn{h}{qc}{half}", tag=f"avn{half}"
                        )
                        nc.vector.tensor_mul(avn, av_ps[half], rb_sb)
                        rr = r0 + half * 128
                        nc.sync.dma_start(avTn[rr : rr + 128, qs], avn)

        # =========== Phase F: fc + residual + layernorm ===========
        with ExitStack() as ph:
            wp = ph.enter_context(tc.tile_pool(name="w_fc", bufs=1))
            ap_ = ph.enter_context(tc.tile_pool(name="avt", bufs=1))
            wk = ph.enter_context(tc.tile_pool(name="ln_work", bufs=3))
            ps = ph.enter_context(tc.tile_pool(name="fc_ps", bufs=2, space="PSUM"))
            w_sb = wp.tile([128, DT, D], bf16, name="w_sb_fc")
            nc.sync.dma_start(w_sb, WfcT.rearrange("(t p) n -> p t n", p=128))
            av_sb = ap_.tile([128, DT, L], bf16, name="av_sb")
            nc.sync.dma_start(av_sb, avTn.rearrange("(t p) q -> p t q", p=128))

            sd = nc.vector.BN_STATS_DIM
            for q_t in range(LT):
                qsl = slice(q_t * 128, (q_t + 1) * 128)
                resid = wk.tile([128, D], f32, name=f"res{q_t}", tag="resid")
                nc.sync.dma_start(resid, q_nat[qsl, :])
                fc_ps = [
                    ps.tile([128, 512], f32, name=f"fc{q_t}_{i}", tag=f"fc{i}")
                    for i in range(2)
                ]
                for dc in range(2):
                    for din in range(DT):
                        nc.tensor.matmul(
                            fc_ps[dc],
                            av_sb[:, din, q_t * 128 : (q_t + 1) * 128],
                            w_sb[:, din, dc * 512 : (dc + 1) * 512],
                            start=(din == 0),
                            stop=(din == DT - 1 and not use_bfc),
                        )
                    if use_bfc:
                        nc.tensor.matmul(
                            fc_ps[dc],
                            ones_row,
                            bfc_sb[:, dc * 512 : (dc + 1) * 512],
                            start=False,
                            stop=True,
                        )
                x = wk.tile([128, D], f32, name=f"x{q_t}", tag="x")
                for dc in range(2):
                    nc.vector.tensor_add(
                        x[:, dc * 512 : (dc + 1) * 512],
                        fc_ps[dc],
                        resid[:, dc * 512 : (dc + 1) * 512],
                    )
                stats = wk.tile([128, 2, sd], f32, name=f"stats{q_t}", tag="stats")
                nc.vector.bn_stats(stats[:, 0, :], x[:, 0:512])
                nc.vector.bn_stats(stats[:, 1, :], x[:, 512:1024])
                mv = wk.tile([128, nc.vector.BN_AGGR_DIM], f32, name=f"mv{q_t}", tag="mv")
                nc.vector.bn_aggr(mv, stats)
                rstd = wk.tile([128, 1], f32, name=f"rstd{q_t}", tag="rstd")
                nc.scalar.activation(rstd, mv[:, 1:2], AF.Sqrt, bias=eps_t)
                nc.vector.reciprocal(rstd, rstd)
                y = wk.tile([128, D], f32, name=f"y{q_t}", tag="y")
                nc.vector.tensor_scalar(
                    out=y,
                    in0=x,
                    scalar1=mv[:, 0:1],
                    scalar2=rstd,
                    op0=mybir.AluOpType.subtract,
                    op1=mybir.AluOpType.mult,
                )
                if use_gamma:
                    nc.vector.tensor_mul(y, y, gamma_bc)
                if use_beta:
                    nc.vector.tensor_add(y, y, beta_bc)
                nc.sync.dma_start(out[qsl, :], y)

    nc.compile()
    return nc


def TileCtx(nc):
    return tile.TileContext(nc)


_cache = {}


def _get_program(flags):
    key = tuple(sorted(flags.items()))
    if key not in _cache:
        _cache[key] = build(**flags)
    return _cache[key]


def kernel(**inputs):
    global LAST_RESULT
    q = np.asarray(inputs["q"], dtype=np.float32)
    k = np.asarray(inputs["k"], dtype=np.float32)
    v = np.asarray(inputs["v"], dtype=np.float32)
    mask = np.asarray(inputs["mask"])
    Wq = np.asarray(inputs["Wq"], dtype=np.float32)
    bq = np.asarray(inputs["bq"], dtype=np.float32)
    Wk = np.asarray(inputs["Wk"], dtype=np.float32)
    bk = np.asarray(inputs["bk"], dtype=np.float32)
    Wv = np.asarray(inputs["Wv"], dtype=np.float32)
    bv = np.asarray(inputs["bv"], dtype=np.float32)
    Wfc = np.asarray(inputs["Wfc"], dtype=np.float32)
    bfc = np.asarray(inputs["bfc"], dtype=np.float32)
    gamma = np.asarray(inputs["gamma"], dtype=np.float32)
    beta = np.asarray(inputs["beta"], dtype=np.float32)

    flags = dict(
        use_bqk=bool(np.any(bq) or np.any(bk)),
        use_bv=bool(np.any(bv)),
        use_bfc=bool(np.any(bfc)),
        use_gamma=bool(np.any(gamma != 1.0)),
        use_beta=bool(np.any(beta)),
    )
    nc = _get_program(flags)

    WqT = Wq.T.astype(ml_dtypes.bfloat16)
    WkT = Wk.T.astype(ml_dtypes.bfloat16)
    WvT = Wv.T.astype(ml_dtypes.bfloat16)
    WfcT = Wfc.T.astype(ml_dtypes.bfloat16)

    neg = np.array(MASK_NEG, dtype=ml_dtypes.bfloat16)
    zero = np.array(0.0, dtype=ml_dtypes.bfloat16)

    shared = dict(WqT=WqT, WkT=WkT, WvT=WvT, WfcT=WfcT)
    if flags["use_bqk"]:
        shared["bq_cols"] = np.ascontiguousarray(bq.reshape(DT, 128).T)
        shared["bk_cols"] = np.ascontiguousarray(bk.reshape(DT, 128).T)
    if flags["use_bv"]:
        shared["bv_row"] = bv.reshape(1, D)
    if flags["use_bfc"]:
        shared["bfc_row"] = bfc.reshape(1, D)
    if flags["use_gamma"]:
        shared["gamma_row"] = gamma.reshape(1, D)
    if flags["use_beta"]:
        shared["beta_row"] = beta.reshape(1, D)

    in_maps = []
    for b in range(B):
        m = dict(shared)
        m["qT"] = q[b].T.astype(ml_dtypes.bfloat16)
        m["kT"] = k[b].T.astype(ml_dtypes.bfloat16)
        m["vT"] = v[b].T.astype(ml_dtypes.bfloat16)
        m["q_nat"] = np.ascontiguousarray(q[b])
        m["maskbT"] = np.where(mask[b].T, neg, zero)
        in_maps.append(m)

    LAST_RESULT = run_bass_kernel_spmd(nc, in_maps, core_ids=list(range(B)))
    return np.stack([r["out"] for r in LAST_RESULT.results], axis=0)


# revision 6
# speedup vs baseline: 2.9182x; 1.2416x over previous
"""Trainium2 Bass kernel for a full MHA block (proj -> masked softmax attention
-> fc -> residual -> layernorm), data-parallel over batch across 8 NeuronCores.

Layout strategy (per core, one batch element):
  - Host pre-transposes weights (W.T) and activations (q.T/k.T/v.T) so every
    matmul contraction dim lands on SBUF partitions with zero on-chip
    transposes.
  - Scores are computed *transposed* (S.T[lk, q]) so that the attention
    probabilities are directly usable as the moving operand of the attn@v
    matmul (contraction over lk = partitions).
  - Softmax without max-subtraction: raw scores are bounded (|S|/16 < ~10),
    masked entries get -2^30 added pre-exp so exp underflows to exactly 0.
    Fully-masked rows then produce sum==0 -> recip(sum+1e-30) finite -> attn
    row exactly 0, matching the reference's NaN-fix.
  - Sum-of-exp over partitions via a ones-vector matmul on the PE; the
    reciprocal row is broadcast back to 128 partitions with a K=1 matmul.
"""

import sys

if "/opt/trn_rl_repo" not in sys.path:
    sys.path.insert(0, "/opt/trn_rl_repo")

from contextlib import ExitStack

import ml_dtypes
import numpy as np

import concourse.bass as bass
import concourse.tile as tile
from concourse import bacc, mybir
from concourse.bass_utils import run_bass_kernel_spmd

B, L, D, H = 8, 2048, 1024, 4
DK = D // H  # 256
LT = L // 128  # 16 l-tiles of 128
DT = D // 128  # 8 d-tiles of 128
NQ = 4  # attention q chunks
QW = L // NQ  # 512 q columns per chunk
EPS = 1e-5
INV_TEMP = 1.0 / 16.0  # 1/sqrt(DK)
MASK_NEG = -float(2**30)

f32 = mybir.dt.float32
bf16 = mybir.dt.bfloat16

AF = mybir.ActivationFunctionType

# set by kernel() after each run; test.py reads it for the HW time
LAST_RESULT = None


def build(use_bqk, use_bv, use_bfc, use_gamma, use_beta):
    nc = bacc.Bacc("TRN2", target_bir_lowering=False, debug=False, num_devices=B)

    # ---- per-core I/O (full tensors for one batch element) ----
    qT = nc.dram_tensor("qT", [D, L], bf16, kind="ExternalInput")
    kT = nc.dram_tensor("kT", [D, L], bf16, kind="ExternalInput")
    vT = nc.dram_tensor("vT", [D, L], bf16, kind="ExternalInput")
    q_nat = nc.dram_tensor("q_nat", [L, D], f32, kind="ExternalInput")
    maskbT = nc.dram_tensor("maskbT", [L, L], bf16, kind="ExternalInput")
    WqT = nc.dram_tensor("WqT", [D, D], bf16, kind="ExternalInput")
    WkT = nc.dram_tensor("WkT", [D, D], bf16, kind="ExternalInput")
    WvT = nc.dram_tensor("WvT", [D, D], bf16, kind="ExternalInput")
    WfcT = nc.dram_tensor("WfcT", [D, D], bf16, kind="ExternalInput")
    if use_bqk:
        # bias for dout tile j lives in column j as a [128,1] per-partition vec
        bq_cols = nc.dram_tensor("bq_cols", [128, DT], f32, kind="ExternalInput")
        bk_cols = nc.dram_tensor("bk_cols", [128, DT], f32, kind="ExternalInput")
    if use_bv:
        bv_row = nc.dram_tensor("bv_row", [1, D], f32, kind="ExternalInput")
    if use_bfc:
        bfc_row = nc.dram_tensor("bfc_row", [1, D], f32, kind="ExternalInput")
    if use_gamma:
        gamma_row = nc.dram_tensor("gamma_row", [1, D], f32, kind="ExternalInput")
    if use_beta:
        beta_row = nc.dram_tensor("beta_row", [1, D], f32, kind="ExternalInput")
    out = nc.dram_tensor("out", [L, D], f32, kind="ExternalOutput")

    with TileCtx(nc) as tc, ExitStack() as top:
        dram = top.enter_context(tc.tile_pool(name="dram", bufs=1, space="DRAM"))
        qpT = dram.tile([D, L], bf16, name="qpT")  # [dout, l]
        kpT = dram.tile([D, L], bf16, name="kpT")  # [dout, l]
        vp = dram.tile([L, D], bf16, name="vp")  # [l, dout]
        avTn = dram.tile([D, L], bf16, name="avTn")  # [dconcat, q] normalized

        singles = top.enter_context(tc.tile_pool(name="singles", bufs=1))
        ones_col = singles.tile([128, 1], bf16, name="ones_col")
        nc.vector.memset(ones_col, 1.0)
        ones_row = singles.tile([1, 128], f32, name="ones_row")
        nc.vector.memset(ones_row, 1.0)
        eps_t = singles.tile([128, 1], f32, name="eps_t")
        nc.vector.memset(eps_t, EPS)
        ident = singles.tile([128, 128], bf16, name="ident")
        nc.gpsimd.memset(ident, 0.0)
        nc.gpsimd.affine_select(
            out=ident,
            in_=ident,
            compare_op=mybir.AluOpType.not_equal,
            fill=1.0,
            base=0,
            pattern=[[-1, 128]],
            channel_multiplier=1,
        )
        if use_bqk:
            bq_sb = singles.tile([128, DT], f32, name="bq_sb")
            nc.sync.dma_start(bq_sb, bq_cols[:, :])
            bk_sb = singles.tile([128, DT], f32, name="bk_sb")
            nc.sync.dma_start(bk_sb, bk_cols[:, :])
        if use_bv:
            bv_sb = singles.tile([1, D], f32, name="bv_sb")
            nc.sync.dma_start(bv_sb, bv_row[:, :])
        if use_bfc:
            bfc_sb = singles.tile([1, D], f32, name="bfc_sb")
            nc.sync.dma_start(bfc_sb, bfc_row[:, :])
        if use_gamma:
            gamma_bc = singles.tile([128, D], f32, name="gamma_bc")
            g = gamma_row[:, :]
            nc.gpsimd.dma_start(
                out=gamma_bc,
                in_=bass.AP(tensor=g.tensor, offset=g.offset, ap=[[0, 128], [1, D]]),
            )
        if use_beta:
            beta_bc = singles.tile([128, D], f32, name="beta_bc")
            bt = beta_row[:, :]
            nc.gpsimd.dma_start(
                out=beta_bc,
                in_=bass.AP(tensor=bt.tensor, offset=bt.offset, ap=[[0, 128], [1, D]]),
            )

        # =========== Phase P: projections ===========
        # q/k: out_pT[dout, l] = W @ x.T   (lhsT = W.T[din,dout], rhs = x.T[din,l])
        for name, WT_d, xT_d, outT, bias_sb in (
            ("q", WqT, qT, qpT, bq_sb if use_bqk else None),
            ("k", WkT, kT, kpT, bk_sb if use_bqk else None),
        ):
            with ExitStack() as ph:
                wp = ph.enter_context(tc.tile_pool(name=f"w_{name}", bufs=1))
                xp = ph.enter_context(tc.tile_pool(name=f"x_{name}", bufs=1))
                ev = ph.enter_context(tc.tile_pool(name=f"ev_{name}", bufs=3))
                ps = ph.enter_context(
                    tc.tile_pool(name=f"ps_{name}", bufs=2, space="PSUM")
                )
                w_sb = wp.tile([128, DT, D], bf16, name=f"w_sb_{name}")
                nc.sync.dma_start(w_sb, WT_d.rearrange("(t p) n -> p t n", p=128))
                x_sb = xp.tile([128, DT, L], bf16, name=f"x_sb_{name}")
                nc.sync.dma_start(x_sb, xT_d.rearrange("(t p) l -> p t l", p=128))
                for dout_t in range(DT):
                    pst = [
                        ps.tile([128, QW], f32, name=f"pp{name}{dout_t}_{i}", tag=f"pp{i}")
                        for i in range(NQ)
                    ]
                    for din in range(DT):
                        lhsT = w_sb[:, din, dout_t * 128 : (dout_t + 1) * 128]
                        for qc in range(NQ):
                            nc.tensor.matmul(
                                pst[qc],
                                lhsT,
                                x_sb[:, din, qc * QW : (qc + 1) * QW],
                                start=(din == 0),
                                stop=(din == DT - 1),
                            )
                    evt = ev.tile([128, L], bf16, name=f"evt_{name}", tag="evt")
                    for qc in range(NQ):
                        dst = evt[:, qc * QW : (qc + 1) * QW]
                        if bias_sb is not None:
                            nc.scalar.activation(
                                dst,
                                pst[qc],
                                AF.Identity,
                                bias=bias_sb[:, dout_t : dout_t + 1],
                            )
                        else:
                            nc.scalar.copy(dst, pst[qc])
                    nc.sync.dma_start(
                        outT[dout_t * 128 : (dout_t + 1) * 128, :], evt
                    )

        # v: vp[l, dout] = x.T.T @ W.T   (lhsT = v.T[din, l], rhs = W.T[din, dout])
        with ExitStack() as ph:
            wp = ph.enter_context(tc.tile_pool(name="w_v", bufs=1))
            xp = ph.enter_context(tc.tile_pool(name="x_v", bufs=1))
            ev = ph.enter_context(tc.tile_pool(name="ev_v", bufs=3))
            ps = ph.enter_context(tc.tile_pool(name="ps_v", bufs=2, space="PSUM"))
            w_sb = wp.tile([128, DT, D], bf16, name="w_sb_v")
            nc.sync.dma_start(w_sb, WvT.rearrange("(t p) n -> p t n", p=128))
            x_sb = xp.tile([128, DT, L], bf16, name="x_sb_v")
            nc.sync.dma_start(x_sb, vT.rearrange("(t p) l -> p t l", p=128))
            for l_t in range(LT):
                pst = [
                    ps.tile([128, 512], f32, name=f"ppv{l_t}_{i}", tag=f"ppv{i}")
                    for i in range(2)
                ]
                for din in range(DT):
                    lhsT = x_sb[:, din, l_t * 128 : (l_t + 1) * 128]
                    for dc in range(2):
                        nc.tensor.matmul(
                            pst[dc],
                            lhsT,
                            w_sb[:, din, dc * 512 : (dc + 1) * 512],
                            start=(din == 0),
                            stop=(din == DT - 1 and not use_bv),
                        )
                if use_bv:
                    for dc in range(2):
                        nc.tensor.matmul(
                            pst[dc],
                            ones_row,
                            bv_sb[:, dc * 512 : (dc + 1) * 512],
                            start=False,
                            stop=True,
                        )
                evt = ev.tile([128, D], bf16, name="evt_v", tag="evtv")
                for dc in range(2):
                    nc.scalar.copy(evt[:, dc * 512 : (dc + 1) * 512], pst[dc])
                nc.sync.dma_start(vp[l_t * 128 : (l_t + 1) * 128, :], evt)

        # =========== Phase A: attention per head ===========
        with ExitStack() as ph:
            mp = ph.enter_context(tc.tile_pool(name="maskp", bufs=1))
            mask_sb = mp.tile([128, LT, L], bf16, name="mask_sb")
            nc.sync.dma_start(mask_sb, maskbT.rearrange("(t p) q -> p t q", p=128))

            hp = ph.enter_context(tc.tile_pool(name="headp", bufs=2))
            wk = ph.enter_context(tc.tile_pool(name="attn_work", bufs=3))
            ps = ph.enter_context(tc.tile_pool(name="attn_ps", bufs=1, space="PSUM"))

            for h in range(H):
                r0 = h * DK
                kp_sb = hp.tile([128, 2, L], bf16, name=f"kp_sb{h}", tag="kp")
                nc.sync.dma_start(
                    kp_sb, kpT[r0 : r0 + DK, :].rearrange("(c p) q -> p c q", p=128)
                )
                qp_sb = hp.tile([128, 2, L], bf16, name=f"qp_sb{h}", tag="qp")
                nc.sync.dma_start(
                    qp_sb, qpT[r0 : r0 + DK, :].rearrange("(c p) q -> p c q", p=128)
                )
                vp_sb = hp.tile([128, LT, DK], bf16, name=f"vp_sb{h}", tag="vph")
                nc.sync.dma_start(
                    vp_sb,
                    vp[:, r0 : r0 + DK].rearrange("(t p) d -> p t d", p=128),
                )
                for qc in range(NQ):
                    qs = slice(qc * QW, (qc + 1) * QW)
                    av_ps = [
                        ps.tile([128, QW], f32, name=f"av{h}_{qc}_{i}", tag=f"av{i}", bufs=2)
                        for i in range(2)
                    ]
                    sum_ps = ps.tile([1, QW], f32, name=f"sum{h}_{qc}", tag="sum")
                    for lk in range(LT):
                        st_ps = ps.tile(
                            [128, QW], f32, name=f"st{h}_{qc}_{lk}", tag="st", bufs=2
                        )
                        nc.tensor.matmul(
                            st_ps,
                            kp_sb[:, 0, lk * 128 : (lk + 1) * 128],
                            qp_sb[:, 0, qs],
                            start=True,
                            stop=False,
                        )
                        nc.tensor.matmul(
                            st_ps,
                            kp_sb[:, 1, lk * 128 : (lk + 1) * 128],
                            qp_sb[:, 1, qs],
                            start=False,
                            stop=False,
                        )
                        nc.tensor.matmul(
                            st_ps,
                            ident,
                            mask_sb[:, lk, qs],
                            start=False,
                            stop=True,
                        )
                        pt = wk.tile([128, QW], bf16, name=f"pt{h}{qc}{lk}", tag="pt", bufs=4)
                        nc.scalar.activation(pt, st_ps, AF.Exp, scale=INV_TEMP)
                        nc.tensor.matmul(
                            av_ps[0],
                            vp_sb[:, lk, 0:128],
                            pt,
                            start=(lk == 0),
                            stop=(lk == LT - 1),
                        )
                        nc.tensor.matmul(
                            av_ps[1],
                            vp_sb[:, lk, 128:256],
                            pt,
                            start=(lk == 0),
                            stop=(lk == LT - 1),
                        )
                        nc.tensor.matmul(
                            sum_ps,
                            ones_col,
                            pt,
                            start=(lk == 0),
                            stop=(lk == LT - 1),
                        )
                    srow = wk.tile([1, QW], f32, name=f"srow{h}{qc}", tag="srow")
                    nc.vector.tensor_scalar_add(srow, sum_ps, 1e-30)
                    nc.vector.reciprocal_approx_fast(srow, srow)
                    rb_ps = ps.tile([128, QW], f32, name=f"rb{h}_{qc}", tag="rb")
                    nc.tensor.matmul(rb_ps, ones_row, srow, start=True, stop=True)
                    rb_sb = wk.tile([128, QW], f32, name=f"rbs{h}{qc}", tag="rbs")
                    nc.scalar.copy(rb_sb, rb_ps)
                    for half in range(2):
                        avn = wk.tile(
                            [128, QW], bf16, name=f"avn{h}{qc}{half}", tag=f"avn{half}"
                        )
                        nc.vector.tensor_mul(avn, av_ps[half], rb_sb)
                        rr = r0 + half * 128
                        nc.sync.dma_start(avTn[rr : rr + 128, qs], avn)

        # =========== Phase F: fc + residual + layernorm ===========
        with ExitStack() as ph:
            wp = ph.enter_context(tc.tile_pool(name="w_fc", bufs=1))
            ap_ = ph.enter_context(tc.tile_pool(name="avt", bufs=1))
            wk = ph.enter_context(tc.tile_pool(name="ln_work", bufs=3))
            ps = ph.enter_context(tc.tile_pool(name="fc_ps", bufs=2, space="PSUM"))
            w_sb = wp.tile([128, DT, D], bf16, name="w_sb_fc")
            nc.sync.dma_start(w_sb, WfcT.rearrange("(t p) n -> p t n", p=128))
            av_sb = ap_.tile([128, DT, L], bf16, name="av_sb")
            nc.sync.dma_start(av_sb, avTn.rearrange("(t p) q -> p t q", p=128))

            sd = nc.vector.BN_STATS_DIM
            for q_t in range(LT):
                qsl = slice(q_t * 128, (q_t + 1) * 128)
                resid = wk.tile([128, D], f32, name=f"res{q_t}", tag="resid")
                nc.sync.dma_start(resid, q_nat[qsl, :])
                fc_ps = [
                    ps.tile([128, 512], f32, name=f"fc{q_t}_{i}", tag=f"fc{i}")
                    for i in range(2)
                ]
                for dc in range(2):
                    for din in range(DT):
                        nc.tensor.matmul(
                            fc_ps[dc],
                            av_sb[:, din, q_t * 128 : (q_t + 1) * 128],
                            w_sb[:, din, dc * 512 : (dc + 1) * 512],
                            start=(din == 0),
                            stop=(din == DT - 1 and not use_bfc),
                        )
                    if use_bfc:
                        nc.tensor.matmul(
                            fc_ps[dc],
                            ones_row,
                            bfc_sb[:, dc * 512 : (dc + 1) * 512],
                            start=False,
                            stop=True,
                        )
                x = wk.tile([128, D], f32, name=f"x{q_t}", tag="x")
                for dc in range(2):
                    nc.vector.tensor_add(
                        x[:, dc * 512 : (dc + 1) * 512],
                        fc_ps[dc],
                        resid[:, dc * 512 : (dc + 1) * 512],
                    )
                stats = wk.tile([128, 2, sd], f32, name=f"stats{q_t}", tag="stats")
                nc.vector.bn_stats(stats[:, 0, :], x[:, 0:512])
                nc.vector.bn_stats(stats[:, 1, :], x[:, 512:1024])
                mv = wk.tile([128, nc.vector.BN_AGGR_DIM], f32, name=f"mv{q_t}", tag="mv")
                nc.vector.bn_aggr(mv, stats)
                rstd = wk.tile([128, 1], f32, name=f"rstd{q_t}", tag="rstd")
                nc.scalar.activation(rstd, mv[:, 1:2], AF.Sqrt, bias=eps_t)
                nc.vector.reciprocal(rstd, rstd)
                y = wk.tile([128, D], f32, name=f"y{q_t}", tag="y")
                nc.vector.tensor_scalar(
                    out=y,
                    in0=x,
                    scalar1=mv[:, 0:1],
                    scalar2=rstd,
                    op0=mybir.AluOpType.subtract,
                    op1=mybir.AluOpType.mult,
                )
                if use_gamma:
                    nc.vector.tensor_mul(y, y, gamma_bc)
                if use_beta:
                    nc.vector.tensor_add(y, y, beta_bc)
                nc.sync.dma_start(out[qsl, :], y)

    nc.compile()
    return nc


def TileCtx(nc):
    return tile.TileContext(nc)


_cache = {}


def _get_program(flags):
    key = tuple(sorted(flags.items()))
    if key not in _cache:
        _cache[key] = build(**flags)
    return _cache[key]


def kernel(**inputs):
    global LAST_RESULT
    q = np.asarray(inputs["q"], dtype=np.float32)
    k = np.asarray(inputs["k"], dtype=np.float32)
    v = np.asarray(inputs["v"], dtype=np.float32)
    mask = np.asarray(inputs["mask"])
    Wq = np.asarray(inputs["Wq"], dtype=np.float32)
    bq = np.asarray(inputs["bq"], dtype=np.float32)
    Wk = np.asarray(inputs["Wk"], dtype=np.float32)
    bk = np.asarray(inputs["bk"], dtype=np.float32)
    Wv = np.asarray(inputs["Wv"], dtype=np.float32)
    bv = np.asarray(inputs["bv"], dtype=np.float32)
    Wfc = np.asarray(inputs["Wfc"], dtype=np.float32)
    bfc = np.asarray(inputs["bfc"], dtype=np.float32)
    gamma = np.asarray(inputs["gamma"], dtype=np.float32)
    beta = np.asarray(inputs["beta"], dtype=np.float32)

    flags = dict(
        use_bqk=bool(np.any(bq) or np.any(bk)),
        use_bv=bool(np.any(bv)),
        use_bfc=bool(np.any(bfc)),
        use_gamma=bool(np.any(gamma != 1.0)),
        use_beta=bool(np.any(beta)),
    )
    nc = _get_program(flags)

    WqT = Wq.T.astype(ml_dtypes.bfloat16)
    WkT = Wk.T.astype(ml_dtypes.bfloat16)
    WvT = Wv.T.astype(ml_dtypes.bfloat16)
    WfcT = Wfc.T.astype(ml_dtypes.bfloat16)

    neg = np.array(MASK_NEG, dtype=ml_dtypes.bfloat16)
    zero = np.array(0.0, dtype=ml_dtypes.bfloat16)

    shared = dict(WqT=WqT, WkT=WkT, WvT=WvT, WfcT=WfcT)
    if flags["use_bqk"]:
        shared["bq_cols"] = np.ascontiguousarray(bq.reshape(DT, 128).T)
        shared["bk_cols"] = np.ascontiguousarray(bk.reshape(DT, 128).T)
    if flags["use_bv"]:
        shared["bv_row"] = bv.reshape(1, D)
    if flags["use_bfc"]:
        shared["bfc_row"] = bfc.reshape(1, D)
    if flags["use_gamma"]:
        shared["gamma_row"] = gamma.reshape(1, D)
    if flags["use_beta"]:
        shared["beta_row"] = beta.reshape(1, D)

    in_maps = []
    for b in range(B):
        m = dict(shared)
        m["qT"] = q[b].T.astype(ml_dtypes.bfloat16)
        m["kT"] = k[b].T.astype(ml_dtypes.bfloat16)
        m["vT"] = v[b].T.astype(ml_dtypes.bfloat16)
        m["q_nat"] = np.ascontiguousarray(q[b])
        m["maskbT"] = np.where(mask[b].T, neg, zero)
        in_maps.append(m)

    LAST_RESULT = run_bass_kernel_spmd(nc, in_maps, core_ids=list(range(B)))
    return np.stack([r["out"] for r in LAST_RESULT.results], axis=0)


# revision 7
# speedup vs baseline: 2.9487x; 1.0105x over previous
"""Trainium2 Bass kernel for a full MHA block (proj -> masked softmax attention
-> fc -> residual -> layernorm), data-parallel over batch across 8 NeuronCores.

Layout strategy (per core, one batch element):
  - Host pre-transposes weights (W.T) and activations (q.T/k.T/v.T) so every
    matmul contraction dim lands on SBUF partitions with zero on-chip
    transposes.
  - Scores are computed *transposed* (S.T[lk, q]) so that the attention
    probabilities are directly usable as the moving operand of the attn@v
    matmul (contraction over lk = partitions).
  - Softmax without max-subtraction: raw scores are bounded (|S|/16 < ~10),
    masked entries get -2^30 added pre-exp so exp underflows to exactly 0.
    Fully-masked rows then produce sum==0 -> recip(sum+1e-30) finite -> attn
    row exactly 0, matching the reference's NaN-fix.
  - Sum-of-exp over partitions via a ones-vector matmul on the PE; the
    reciprocal row is broadcast back to 128 partitions with a K=1 matmul.
"""

import sys

if "/opt/trn_rl_repo" not in sys.path:
    sys.path.insert(0, "/opt/trn_rl_repo")

from contextlib import ExitStack

import ml_dtypes
import numpy as np

import concourse.bass as bass
import concourse.tile as tile
from concourse import bacc, mybir
from concourse.bass_utils import run_bass_kernel_spmd

B, L, D, H = 8, 2048, 1024, 4
DK = D // H  # 256
LT = L // 128  # 16 l-tiles of 128
DT = D // 128  # 8 d-tiles of 128
NQ = 4  # attention q chunks
QW = L // NQ  # 512 q columns per chunk
EPS = 1e-5
INV_TEMP = 1.0 / 16.0  # 1/sqrt(DK)
MASK_NEG = -float(2**30)

f32 = mybir.dt.float32
bf16 = mybir.dt.bfloat16

AF = mybir.ActivationFunctionType

# set by kernel() after each run; test.py reads it for the HW time
LAST_RESULT = None


def build(use_bqk, use_bv, use_bfc, use_gamma, use_beta):
    nc = bacc.Bacc("TRN2", target_bir_lowering=False, debug=False, num_devices=B)

    # ---- per-core I/O (full tensors for one batch element) ----
    qT = nc.dram_tensor("qT", [D, L], bf16, kind="ExternalInput")
    kT = nc.dram_tensor("kT", [D, L], bf16, kind="ExternalInput")
    vT = nc.dram_tensor("vT", [D, L], bf16, kind="ExternalInput")
    q_nat = nc.dram_tensor("q_nat", [L, D], f32, kind="ExternalInput")
    maskbT = nc.dram_tensor("maskbT", [L, L], bf16, kind="ExternalInput")
    WqT = nc.dram_tensor("WqT", [D, D], bf16, kind="ExternalInput")
    WkT = nc.dram_tensor("WkT", [D, D], bf16, kind="ExternalInput")
    WvT = nc.dram_tensor("WvT", [D, D], bf16, kind="ExternalInput")
    WfcT = nc.dram_tensor("WfcT", [D, D], bf16, kind="ExternalInput")
    if use_bqk:
        # bias for dout tile j lives in column j as a [128,1] per-partition vec
        bq_cols = nc.dram_tensor("bq_cols", [128, DT], f32, kind="ExternalInput")
        bk_cols = nc.dram_tensor("bk_cols", [128, DT], f32, kind="ExternalInput")
    if use_bv:
        bv_row = nc.dram_tensor("bv_row", [1, D], f32, kind="ExternalInput")
    if use_bfc:
        bfc_row = nc.dram_tensor("bfc_row", [1, D], f32, kind="ExternalInput")
    if use_gamma:
        gamma_row = nc.dram_tensor("gamma_row", [1, D], f32, kind="ExternalInput")
    if use_beta:
        beta_row = nc.dram_tensor("beta_row", [1, D], f32, kind="ExternalInput")
    out = nc.dram_tensor("out", [L, D], f32, kind="ExternalOutput")

    with TileCtx(nc) as tc, ExitStack() as top:
        dram = top.enter_context(tc.tile_pool(name="dram", bufs=1, space="DRAM"))
        qpT = dram.tile([D, L], bf16, name="qpT")  # [dout, l]
        kpT = dram.tile([D, L], bf16, name="kpT")  # [dout, l]

        bigs = top.enter_context(tc.tile_pool(name="bigs", bufs=1))
        vp_big = bigs.tile([128, LT, D], bf16, name="vp_big")  # [l-part, lt, dout]
        avTn_big = bigs.tile([128, DT, L], bf16, name="avTn_big")  # [dchunk, t, q]
        maskp = top.enter_context(tc.tile_pool(name="maskp", bufs=1))
        mask_sb = maskp.tile([128, LT, L], bf16, name="mask_sb")
        nc.sync.dma_start(mask_sb, maskbT.rearrange("(t p) q -> p t q", p=128))

        singles = top.enter_context(tc.tile_pool(name="singles", bufs=1))
        ones_col = singles.tile([128, 1], bf16, name="ones_col")
        nc.vector.memset(ones_col, 1.0)
        ones_row = singles.tile([1, 128], f32, name="ones_row")
        nc.vector.memset(ones_row, 1.0)
        eps_t = singles.tile([128, 1], f32, name="eps_t")
        nc.vector.memset(eps_t, EPS)
        ident = singles.tile([128, 128], bf16, name="ident")
        nc.gpsimd.memset(ident, 0.0)
        nc.gpsimd.affine_select(
            out=ident,
            in_=ident,
            compare_op=mybir.AluOpType.not_equal,
            fill=1.0,
            base=0,
            pattern=[[-1, 128]],
            channel_multiplier=1,
        )
        if use_bqk:
            bq_sb = singles.tile([128, DT], f32, name="bq_sb")
            nc.sync.dma_start(bq_sb, bq_cols[:, :])
            bk_sb = singles.tile([128, DT], f32, name="bk_sb")
            nc.sync.dma_start(bk_sb, bk_cols[:, :])
        if use_bv:
            bv_sb = singles.tile([1, D], f32, name="bv_sb")
            nc.sync.dma_start(bv_sb, bv_row[:, :])
        if use_bfc:
            bfc_sb = singles.tile([1, D], f32, name="bfc_sb")
            nc.sync.dma_start(bfc_sb, bfc_row[:, :])
        if use_gamma:
            gamma_bc = singles.tile([128, D], f32, name="gamma_bc")
            g = gamma_row[:, :]
            nc.gpsimd.dma_start(
                out=gamma_bc,
                in_=bass.AP(tensor=g.tensor, offset=g.offset, ap=[[0, 128], [1, D]]),
            )
        if use_beta:
            beta_bc = singles.tile([128, D], f32, name="beta_bc")
            bt = beta_row[:, :]
            nc.gpsimd.dma_start(
                out=beta_bc,
                in_=bass.AP(tensor=bt.tensor, offset=bt.offset, ap=[[0, 128], [1, D]]),
            )

        # =========== Phase P: projections ===========
        # q/k: out_pT[dout, l] = W @ x.T   (lhsT = W.T[din,dout], rhs = x.T[din,l])
        for name, WT_d, xT_d, outT, bias_sb in (
            ("q", WqT, qT, qpT, bq_sb if use_bqk else None),
            ("k", WkT, kT, kpT, bk_sb if use_bqk else None),
        ):
            with ExitStack() as ph:
                wp = ph.enter_context(tc.tile_pool(name=f"w_{name}", bufs=1))
                xp = ph.enter_context(tc.tile_pool(name=f"x_{name}", bufs=1))
                ev = ph.enter_context(tc.tile_pool(name=f"ev_{name}", bufs=3))
                ps = ph.enter_context(
                    tc.tile_pool(name=f"ps_{name}", bufs=2, space="PSUM")
                )
                w_sb = wp.tile([128, DT, D], bf16, name=f"w_sb_{name}")
                nc.sync.dma_start(w_sb, WT_d.rearrange("(t p) n -> p t n", p=128))
                x_sb = xp.tile([128, DT, L], bf16, name=f"x_sb_{name}")
                nc.sync.dma_start(x_sb, xT_d.rearrange("(t p) l -> p t l", p=128))
                for dout_t in range(DT):
                    pst = [
                        ps.tile([128, QW], f32, name=f"pp{name}{dout_t}_{i}", tag=f"pp{i}")
                        for i in range(NQ)
                    ]
                    for din in range(DT):
                        lhsT = w_sb[:, din, dout_t * 128 : (dout_t + 1) * 128]
                        for qc in range(NQ):
                            nc.tensor.matmul(
                                pst[qc],
                                lhsT,
                                x_sb[:, din, qc * QW : (qc + 1) * QW],
                                start=(din == 0),
                                stop=(din == DT - 1),
                            )
                    evt = ev.tile([128, L], bf16, name=f"evt_{name}", tag="evt")
                    for qc in range(NQ):
                        dst = evt[:, qc * QW : (qc + 1) * QW]
                        if bias_sb is not None:
                            nc.scalar.activation(
                                dst,
                                pst[qc],
                                AF.Identity,
                                bias=bias_sb[:, dout_t : dout_t + 1],
                            )
                        else:
                            nc.scalar.copy(dst, pst[qc])
                    nc.sync.dma_start(
                        outT[dout_t * 128 : (dout_t + 1) * 128, :], evt
                    )

        # v: vp[l, dout] = x.T.T @ W.T   (lhsT = v.T[din, l], rhs = W.T[din, dout])
        with ExitStack() as ph:
            wp = ph.enter_context(tc.tile_pool(name="w_v", bufs=1))
            xp = ph.enter_context(tc.tile_pool(name="x_v", bufs=1))
            ps = ph.enter_context(tc.tile_pool(name="ps_v", bufs=2, space="PSUM"))
            w_sb = wp.tile([128, DT, D], bf16, name="w_sb_v")
            nc.sync.dma_start(w_sb, WvT.rearrange("(t p) n -> p t n", p=128))
            x_sb = xp.tile([128, DT, L], bf16, name="x_sb_v")
            nc.sync.dma_start(x_sb, vT.rearrange("(t p) l -> p t l", p=128))
            for l_t in range(LT):
                pst = [
                    ps.tile([128, 512], f32, name=f"ppv{l_t}_{i}", tag=f"ppv{i}")
                    for i in range(2)
                ]
                for din in range(DT):
                    lhsT = x_sb[:, din, l_t * 128 : (l_t + 1) * 128]
                    for dc in range(2):
                        nc.tensor.matmul(
                            pst[dc],
                            lhsT,
                            w_sb[:, din, dc * 512 : (dc + 1) * 512],
                            start=(din == 0),
                            stop=(din == DT - 1 and not use_bv),
                        )
                if use_bv:
                    for dc in range(2):
                        nc.tensor.matmul(
                            pst[dc],
                            ones_row,
                            bv_sb[:, dc * 512 : (dc + 1) * 512],
                            start=False,
                            stop=True,
                        )
                for dc in range(2):
                    nc.scalar.copy(
                        vp_big[:, l_t, dc * 512 : (dc + 1) * 512], pst[dc]
                    )

        # =========== Phase A: attention per head ===========
        with ExitStack() as ph:
            hp = ph.enter_context(tc.tile_pool(name="headp", bufs=2))
            wk = ph.enter_context(tc.tile_pool(name="attn_work", bufs=3))
            ps = ph.enter_context(tc.tile_pool(name="attn_ps", bufs=1, space="PSUM"))

            for h in range(H):
                r0 = h * DK
                kp_sb = hp.tile([128, 2, L], bf16, name=f"kp_sb{h}", tag="kp")
                nc.sync.dma_start(
                    kp_sb, kpT[r0 : r0 + DK, :].rearrange("(c p) q -> p c q", p=128)
                )
                qp_sb = hp.tile([128, 2, L], bf16, name=f"qp_sb{h}", tag="qp")
                nc.sync.dma_start(
                    qp_sb, qpT[r0 : r0 + DK, :].rearrange("(c p) q -> p c q", p=128)
                )
                for qc in range(NQ):
                    qs = slice(qc * QW, (qc + 1) * QW)
                    av_ps = [
                        ps.tile([128, QW], f32, name=f"av{h}_{qc}_{i}", tag=f"av{i}", bufs=2)
                        for i in range(2)
                    ]
                    sum_ps = ps.tile([1, QW], f32, name=f"sum{h}_{qc}", tag="sum")
                    for lk in range(LT):
                        st_ps = ps.tile(
                            [128, QW], f32, name=f"st{h}_{qc}_{lk}", tag="st", bufs=2
                        )
                        nc.tensor.matmul(
                            st_ps,
                            kp_sb[:, 0, lk * 128 : (lk + 1) * 128],
                            qp_sb[:, 0, qs],
                            start=True,
                            stop=False,
                        )
                        nc.tensor.matmul(
                            st_ps,
                            kp_sb[:, 1, lk * 128 : (lk + 1) * 128],
                            qp_sb[:, 1, qs],
                            start=False,
                            stop=False,
                        )
                        nc.tensor.matmul(
                            st_ps,
                            ident,
                            mask_sb[:, lk, qs],
                            start=False,
                            stop=True,
                        )
                        pt = wk.tile([128, QW], bf16, name=f"pt{h}{qc}{lk}", tag="pt", bufs=4)
                        nc.scalar.activation(pt, st_ps, AF.Exp, scale=INV_TEMP)
                        nc.tensor.matmul(
                            av_ps[0],
                            vp_big[:, lk, r0 : r0 + 128],
                            pt,
                            start=(lk == 0),
                            stop=(lk == LT - 1),
                        )
                        nc.tensor.matmul(
                            av_ps[1],
                            vp_big[:, lk, r0 + 128 : r0 + 256],
                            pt,
                            start=(lk == 0),
                            stop=(lk == LT - 1),
                        )
                        nc.tensor.matmul(
                            sum_ps,
                            ones_col,
                            pt,
                            start=(lk == 0),
                            stop=(lk == LT - 1),
                        )
                    srow = wk.tile([1, QW], f32, name=f"srow{h}{qc}", tag="srow")
                    nc.vector.tensor_scalar_add(srow, sum_ps, 1e-30)
                    nc.vector.reciprocal_approx_fast(srow, srow)
                    rb_ps = ps.tile([128, QW], f32, name=f"rb{h}_{qc}", tag="rb")
                    nc.tensor.matmul(rb_ps, ones_row, srow, start=True, stop=True)
                    rb_sb = wk.tile([128, QW], f32, name=f"rbs{h}{qc}", tag="rbs")
                    nc.scalar.copy(rb_sb, rb_ps)
                    for half in range(2):
                        nc.vector.tensor_mul(
                            avTn_big[:, 2 * h + half, qs], av_ps[half], rb_sb
                        )

        # =========== Phase F: fc + residual + layernorm ===========
        with ExitStack() as ph:
            wp = ph.enter_context(tc.tile_pool(name="w_fc", bufs=1))
            wk = ph.enter_context(tc.tile_pool(name="ln_work", bufs=3))
            ps = ph.enter_context(tc.tile_pool(name="fc_ps", bufs=2, space="PSUM"))
            w_sb = wp.tile([128, DT, D], bf16, name="w_sb_fc")
            nc.sync.dma_start(w_sb, WfcT.rearrange("(t p) n -> p t n", p=128))

            sd = nc.vector.BN_STATS_DIM
            for q_t in range(LT):
                qsl = slice(q_t * 128, (q_t + 1) * 128)
                resid = wk.tile([128, D], f32, name=f"res{q_t}", tag="resid")
                nc.sync.dma_start(resid, q_nat[qsl, :])
                fc_ps = [
                    ps.tile([128, 512], f32, name=f"fc{q_t}_{i}", tag=f"fc{i}")
                    for i in range(2)
                ]
                for dc in range(2):
                    for din in range(DT):
                        nc.tensor.matmul(
                            fc_ps[dc],
                            avTn_big[:, din, q_t * 128 : (q_t + 1) * 128],
                            w_sb[:, din, dc * 512 : (dc + 1) * 512],
                            start=(din == 0),
                            stop=(din == DT - 1 and not use_bfc),
                        )
                    if use_bfc:
                        nc.tensor.matmul(
                            fc_ps[dc],
                            ones_row,
                            bfc_sb[:, dc * 512 : (dc + 1) * 512],
                            start=False,
                            stop=True,
                        )
                x = wk.tile([128, D], f32, name=f"x{q_t}", tag="x")
                for dc in range(2):
                    nc.vector.tensor_add(
                        x[:, dc * 512 : (dc + 1) * 512],
                        fc_ps[dc],
                        resid[:, dc * 512 : (dc + 1) * 512],
                    )
                stats = wk.tile([128, 2, sd], f32, name=f"stats{q_t}", tag="stats")
                nc.vector.bn_stats(stats[:, 0, :], x[:, 0:512])
                nc.vector.bn_stats(stats[:, 1, :], x[:, 512:1024])
                mv = wk.tile([128, nc.vector.BN_AGGR_DIM], f32, name=f"mv{q_t}", tag="mv")
                nc.vector.bn_aggr(mv, stats)
                rstd = wk.tile([128, 1], f32, name=f"rstd{q_t}", tag="rstd")
                nc.scalar.activation(rstd, mv[:, 1:2], AF.Sqrt, bias=eps_t)
                nc.vector.reciprocal(rstd, rstd)
                y = wk.tile([128, D], f32, name=f"y{q_t}", tag="y")
                nc.vector.tensor_scalar(
                    out=y,
                    in0=x,
                    scalar1=mv[:, 0:1],
                    scalar2=rstd,
                    op0=mybir.AluOpType.subtract,
                    op1=mybir.AluOpType.mult,
                )
                if use_gamma:
                    nc.vector.tensor_mul(y, y, gamma_bc)
                if use_beta:
                    nc.vector.tensor_add(y, y, beta_bc)
                nc.sync.dma_start(out[qsl, :], y)

    nc.compile()
    return nc


def TileCtx(nc):
    return tile.TileContext(nc)


_cache = {}


def _get_program(flags):
    key = tuple(sorted(flags.items()))
    if key not in _cache:
        _cache[key] = build(**flags)
    return _cache[key]


def kernel(**inputs):
    global LAST_RESULT
    q = np.asarray(inputs["q"], dtype=np.float32)
    k = np.asarray(inputs["k"], dtype=np.float32)
    v = np.asarray(inputs["v"], dtype=np.float32)
    mask = np.asarray(inputs["mask"])
    Wq = np.asarray(inputs["Wq"], dtype=np.float32)
    bq = np.asarray(inputs["bq"], dtype=np.float32)
    Wk = np.asarray(inputs["Wk"], dtype=np.float32)
    bk = np.asarray(inputs["bk"], dtype=np.float32)
    Wv = np.asarray(inputs["Wv"], dtype=np.float32)
    bv = np.asarray(inputs["bv"], dtype=np.float32)
    Wfc = np.asarray(inputs["Wfc"], dtype=np.float32)
    bfc = np.asarray(inputs["bfc"], dtype=np.float32)
    gamma = np.asarray(inputs["gamma"], dtype=np.float32)
    beta = np.asarray(inputs["beta"], dtype=np.float32)

    flags = dict(
        use_bqk=bool(np.any(bq) or np.any(bk)),
        use_bv=bool(np.any(bv)),
        use_bfc=bool(np.any(bfc)),
        use_gamma=bool(np.any(gamma != 1.0)),
        use_beta=bool(np.any(beta)),
    )
    nc = _get_program(flags)

    WqT = Wq.T.astype(ml_dtypes.bfloat16)
    WkT = Wk.T.astype(ml_dtypes.bfloat16)
    WvT = Wv.T.astype(ml_dtypes.bfloat16)
    WfcT = Wfc.T.astype(ml_dtypes.bfloat16)

    neg = np.array(MASK_NEG, dtype=ml_dtypes.bfloat16)
    zero = np.array(0.0, dtype=ml_dtypes.bfloat16)

    shared = dict(WqT=WqT, WkT=WkT, WvT=WvT, WfcT=WfcT)
    if flags["use_bqk"]:
        shared["bq_cols"] = np.ascontiguousarray(bq.reshape(DT, 128).T)
        shared["bk_cols"] = np.ascontiguousarray(bk.reshape(DT, 128).T)
    if flags["use_bv"]:
        shared["bv_row"] = bv.reshape(1, D)
    if flags["use_bfc"]:
        shared["bfc_row"] = bfc.reshape(1, D)
    if flags["use_gamma"]:
        shared["gamma_row"] = gamma.reshape(1, D)
    if flags["use_beta"]:
        shared["beta_row"] = beta.reshape(1, D)

    in_maps = []
    for b in range(B):
        m = dict(shared)
        m["qT"] = q[b].T.astype(ml_dtypes.bfloat16)
        m["kT"] = k[b].T.astype(ml_dtypes.bfloat16)
        m["vT"] = v[b].T.astype(ml_dtypes.bfloat16)
        m["q_nat"] = np.ascontiguousarray(q[b])
        m["maskbT"] = np.where(mask[b].T, neg, zero)
        in_maps.append(m)

    LAST_RESULT = run_bass_kernel_spmd(nc, in_maps, core_ids=list(range(B)))
    return np.stack([r["out"] for r in LAST_RESULT.results], axis=0)


# revision 9
# speedup vs baseline: 3.3921x; 1.1504x over previous
"""Trainium2 Bass kernel for a full MHA block (proj -> masked softmax attention
-> fc -> residual -> layernorm), data-parallel over batch across 8 NeuronCores.

Layout strategy (per core, one batch element):
  - Host pre-transposes weights (W.T) and activations (q.T/k.T/v.T) so every
    matmul contraction dim lands on SBUF partitions with zero on-chip
    transposes.
  - Scores are computed *transposed* (S.T[lk, q]) so that the attention
    probabilities are directly usable as the moving operand of the attn@v
    matmul (contraction over lk = partitions).
  - Softmax without max-subtraction: raw scores are bounded (|S|/16 < ~10),
    masked entries get -2^30 added pre-exp so exp underflows to exactly 0.
    Fully-masked rows then produce sum==0 -> recip(sum+1e-30) finite -> attn
    row exactly 0, matching the reference's NaN-fix.
  - Sum-of-exp over partitions via a ones-vector matmul on the PE; the
    reciprocal row is broadcast back to 128 partitions with a K=1 matmul.
"""

import sys

if "/opt/trn_rl_repo" not in sys.path:
    sys.path.insert(0, "/opt/trn_rl_repo")

from contextlib import ExitStack

import ml_dtypes
import numpy as np

import concourse.bass as bass
import concourse.tile as tile
from concourse import bacc, mybir
from concourse.bass_utils import run_bass_kernel_spmd

B, L, D, H = 8, 2048, 1024, 4
DK = D // H  # 256
LT = L // 128  # 16 l-tiles of 128
DT = D // 128  # 8 d-tiles of 128
NQ = 4  # attention q chunks
QW = L // NQ  # 512 q columns per chunk
EPS = 1e-5
INV_TEMP = 1.0 / 16.0  # 1/sqrt(DK)
MASK_NEG = -float(2**30)

f32 = mybir.dt.float32
bf16 = mybir.dt.bfloat16

AF = mybir.ActivationFunctionType

# set by kernel() after each run; test.py reads it for the HW time
LAST_RESULT = None


def build(use_bqk, use_bv, use_bfc, use_gamma, use_beta):
    nc = bacc.Bacc("TRN2", target_bir_lowering=False, debug=False, num_devices=B)

    # ---- per-core I/O (full tensors for one batch element) ----
    qT = nc.dram_tensor("qT", [D, L], bf16, kind="ExternalInput")
    kT = nc.dram_tensor("kT", [D, L], bf16, kind="ExternalInput")
    vT = nc.dram_tensor("vT", [D, L], bf16, kind="ExternalInput")
    q_nat = nc.dram_tensor("q_nat", [L, D], f32, kind="ExternalInput")
    maskbT = nc.dram_tensor("maskbT", [L, L], bf16, kind="ExternalInput")
    WqT = nc.dram_tensor("WqT", [D, D], bf16, kind="ExternalInput")
    WkT = nc.dram_tensor("WkT", [D, D], bf16, kind="ExternalInput")
    WvT = nc.dram_tensor("WvT", [D, D], bf16, kind="ExternalInput")
    WfcT = nc.dram_tensor("WfcT", [D, D], bf16, kind="ExternalInput")
    if use_bqk:
        # bias for dout tile j lives in column j as a [128,1] per-partition vec
        bq_cols = nc.dram_tensor("bq_cols", [128, DT], f32, kind="ExternalInput")
        bk_cols = nc.dram_tensor("bk_cols", [128, DT], f32, kind="ExternalInput")
    if use_bv:
        bv_row = nc.dram_tensor("bv_row", [1, D], f32, kind="ExternalInput")
    if use_bfc:
        bfc_row = nc.dram_tensor("bfc_row", [1, D], f32, kind="ExternalInput")
    if use_gamma:
        gamma_row = nc.dram_tensor("gamma_row", [1, D], f32, kind="ExternalInput")
    if use_beta:
        beta_row = nc.dram_tensor("beta_row", [1, D], f32, kind="ExternalInput")
    out = nc.dram_tensor("out", [L, D], f32, kind="ExternalOutput")

    with TileCtx(nc) as tc, ExitStack() as top:
        dram = top.enter_context(tc.tile_pool(name="dram", bufs=1, space="DRAM"))
        qpT = dram.tile([D, L], bf16, name="qpT")  # [dout, l]
        kpT = dram.tile([D, L], bf16, name="kpT")  # [dout, l]
        srow_d = dram.tile([H * NQ, QW], f32, name="srow_d")  # recip bounce rows

        bigs = top.enter_context(tc.tile_pool(name="bigs", bufs=1))
        vp_big = bigs.tile([128, LT, D], bf16, name="vp_big")  # [l-part, lt, dout]
        avTn_big = bigs.tile([128, DT, L], bf16, name="avTn_big")  # [dchunk, t, q]
        maskp = top.enter_context(tc.tile_pool(name="maskp", bufs=1))
        mask_sb = maskp.tile([128, LT, L], bf16, name="mask_sb")

        singles = top.enter_context(tc.tile_pool(name="singles", bufs=1))
        ones_col = singles.tile([128, 1], bf16, name="ones_col")
        nc.vector.memset(ones_col, 1.0)
        ones_row = singles.tile([1, 128], f32, name="ones_row")
        nc.vector.memset(ones_row, 1.0)
        eps_t = singles.tile([128, 1], f32, name="eps_t")
        nc.vector.memset(eps_t, EPS)
        ident = singles.tile([128, 128], bf16, name="ident")
        nc.gpsimd.memset(ident, 0.0)
        nc.gpsimd.affine_select(
            out=ident,
            in_=ident,
            compare_op=mybir.AluOpType.not_equal,
            fill=1.0,
            base=0,
            pattern=[[-1, 128]],
            channel_multiplier=1,
        )
        if use_bqk:
            bq_sb = singles.tile([128, DT], f32, name="bq_sb")
            nc.sync.dma_start(bq_sb, bq_cols[:, :])
            bk_sb = singles.tile([128, DT], f32, name="bk_sb")
            nc.sync.dma_start(bk_sb, bk_cols[:, :])
        if use_bv:
            bv_sb = singles.tile([1, D], f32, name="bv_sb")
            nc.sync.dma_start(bv_sb, bv_row[:, :])
        if use_bfc:
            bfc_sb = singles.tile([1, D], f32, name="bfc_sb")
            nc.sync.dma_start(bfc_sb, bfc_row[:, :])
        if use_gamma:
            gamma_bc = singles.tile([128, D], f32, name="gamma_bc")
            g = gamma_row[:, :]
            nc.gpsimd.dma_start(
                out=gamma_bc,
                in_=bass.AP(tensor=g.tensor, offset=g.offset, ap=[[0, 128], [1, D]]),
            )
        if use_beta:
            beta_bc = singles.tile([128, D], f32, name="beta_bc")
            bt = beta_row[:, :]
            nc.gpsimd.dma_start(
                out=beta_bc,
                in_=bass.AP(tensor=bt.tensor, offset=bt.offset, ap=[[0, 128], [1, D]]),
            )

        # =========== Phase P: projections ===========
        # q/k: out_pT[dout, l] = W @ x.T  (lhsT = W.T[din,dout], rhs = x.T[din,l])
        # x streamed in l-chunks; w double-buffered so q/k/v overlap.
        mask_r = maskbT.rearrange("(t p) q -> p t q", p=128)
        with ExitStack() as ph:
            wp = ph.enter_context(tc.tile_pool(name="w_pool", bufs=2))
            xp = ph.enter_context(tc.tile_pool(name="x_pool", bufs=3))
            ev = ph.enter_context(tc.tile_pool(name="ev_pool", bufs=3))
            ps = ph.enter_context(tc.tile_pool(name="ps_proj", bufs=1, space="PSUM"))

            first = True
            for name, WT_d, xT_d, outT, bias_sb in (
                ("q", WqT, qT, qpT, bq_sb if use_bqk else None),
                ("k", WkT, kT, kpT, bk_sb if use_bqk else None),
            ):
                w_sb = wp.tile([128, DT, D], bf16, name=f"w_{name}", tag="w")
                nc.sync.dma_start(w_sb, WT_d.rearrange("(t p) n -> p t n", p=128))
                xr = xT_d.rearrange("(t p) l -> p t l", p=128)
                for qc in range(NQ):
                    x_c = xp.tile([128, DT, QW], bf16, name=f"x_{name}{qc}", tag="xc")
                    nc.sync.dma_start(x_c, xr[:, :, qc * QW : (qc + 1) * QW])
                    if first:
                        # prefetch mask behind the first tensor's activations
                        nc.sync.dma_start(
                            mask_sb[:, qc * 4 : (qc + 1) * 4, :],
                            mask_r[:, qc * 4 : (qc + 1) * 4, :],
                        )
                    for dout_t in range(DT):
                        pst = ps.tile(
                            [128, QW], f32, name=f"pp{name}{qc}{dout_t}", tag="pp", bufs=3
                        )
                        for din in range(DT):
                            nc.tensor.matmul(
                                pst,
                                w_sb[:, din, dout_t * 128 : (dout_t + 1) * 128],
                                x_c[:, din, :],
                                start=(din == 0),
                                stop=(din == DT - 1),
                            )
                        evt = ev.tile(
                            [128, QW], bf16, name=f"ev{name}{qc}{dout_t}", tag="evt"
                        )
                        if bias_sb is not None:
                            nc.scalar.activation(
                                evt,
                                pst,
                                AF.Identity,
                                bias=bias_sb[:, dout_t : dout_t + 1],
                            )
                        else:
                            nc.scalar.copy(evt, pst)
                        nc.sync.dma_start(
                            outT[
                                dout_t * 128 : (dout_t + 1) * 128,
                                qc * QW : (qc + 1) * QW,
                            ],
                            evt,
                        )
                first = False

            # v: vp[l, dout] = (v.T).T @ W.T  (lhsT = v.T[din, l], rhs = W.T)
            w_sb = wp.tile([128, DT, D], bf16, name="w_v", tag="w")
            nc.sync.dma_start(w_sb, WvT.rearrange("(t p) n -> p t n", p=128))
            vr = vT.rearrange("(t p) l -> p t l", p=128)
            for g in range(NQ):
                x_c = xp.tile([128, DT, QW], bf16, name=f"x_v{g}", tag="xc")
                nc.sync.dma_start(x_c, vr[:, :, g * QW : (g + 1) * QW])
                for j in range(4):
                    l_t = g * 4 + j
                    pst2 = [
                        ps.tile(
                            [128, 512], f32, name=f"ppv{l_t}_{i}", tag=f"ppv{i}", bufs=2
                        )
                        for i in range(2)
                    ]
                    for din in range(DT):
                        lhsT = x_c[:, din, j * 128 : (j + 1) * 128]
                        for dc in range(2):
                            nc.tensor.matmul(
                                pst2[dc],
                                lhsT,
                                w_sb[:, din, dc * 512 : (dc + 1) * 512],
                                start=(din == 0),
                                stop=(din == DT - 1 and not use_bv),
                            )
                    if use_bv:
                        for dc in range(2):
                            nc.tensor.matmul(
                                pst2[dc],
                                ones_row,
                                bv_sb[:, dc * 512 : (dc + 1) * 512],
                                start=False,
                                stop=True,
                            )
                    for dc in range(2):
                        nc.scalar.copy(
                            vp_big[:, l_t, dc * 512 : (dc + 1) * 512], pst2[dc]
                        )

        # =========== Phase A: attention per head ===========
        with ExitStack() as ph:
            hp = ph.enter_context(tc.tile_pool(name="headp", bufs=2))
            wk = ph.enter_context(tc.tile_pool(name="attn_work", bufs=3))
            ps = ph.enter_context(tc.tile_pool(name="attn_ps", bufs=1, space="PSUM"))

            for h in range(H):
                r0 = h * DK
                kp_sb = hp.tile([128, 2, L], bf16, name=f"kp_sb{h}", tag="kp")
                nc.sync.dma_start(
                    kp_sb, kpT[r0 : r0 + DK, :].rearrange("(c p) q -> p c q", p=128)
                )
                qp_sb = hp.tile([128, 2, L], bf16, name=f"qp_sb{h}", tag="qp")
                nc.sync.dma_start(
                    qp_sb, qpT[r0 : r0 + DK, :].rearrange("(c p) q -> p c q", p=128)
                )
                for qc in range(NQ):
                    qs = slice(qc * QW, (qc + 1) * QW)
                    av_ps = [
                        ps.tile([128, QW], f32, name=f"av{h}_{qc}_{i}", tag=f"av{i}", bufs=2)
                        for i in range(2)
                    ]
                    sum_ps = ps.tile([1, QW], f32, name=f"sum{h}_{qc}", tag="sum")
                    for lk in range(LT):
                        st_ps = ps.tile(
                            [128, QW], f32, name=f"st{h}_{qc}_{lk}", tag="st", bufs=3
                        )
                        nc.tensor.matmul(
                            st_ps,
                            kp_sb[:, 0, lk * 128 : (lk + 1) * 128],
                            qp_sb[:, 0, qs],
                            start=True,
                            stop=False,
                        )
                        nc.tensor.matmul(
                            st_ps,
                            kp_sb[:, 1, lk * 128 : (lk + 1) * 128],
                            qp_sb[:, 1, qs],
                            start=False,
                            stop=False,
                        )
                        nc.tensor.matmul(
                            st_ps,
                            ident,
                            mask_sb[:, lk, qs],
                            start=False,
                            stop=True,
                        )
                        pt = wk.tile([128, QW], bf16, name=f"pt{h}{qc}{lk}", tag="pt", bufs=6)
                        nc.scalar.activation(pt, st_ps, AF.Exp, scale=INV_TEMP)
                        nc.tensor.matmul(
                            av_ps[0],
                            vp_big[:, lk, r0 : r0 + 128],
                            pt,
                            start=(lk == 0),
                            stop=(lk == LT - 1),
                        )
                        nc.tensor.matmul(
                            av_ps[1],
                            vp_big[:, lk, r0 + 128 : r0 + 256],
                            pt,
                            start=(lk == 0),
                            stop=(lk == LT - 1),
                        )
                        nc.tensor.matmul(
                            sum_ps,
                            ones_col,
                            pt,
                            start=(lk == 0),
                            stop=(lk == LT - 1),
                        )
                    srow = wk.tile([1, QW], f32, name=f"srow{h}{qc}", tag="srow")
                    nc.vector.tensor_scalar_add(srow, sum_ps, 1e-30)
                    nc.vector.reciprocal_approx_fast(srow, srow)
                    idx = h * NQ + qc
                    nc.sync.dma_start(srow_d[idx : idx + 1, :], srow)
                    rb_sb = wk.tile([128, QW], f32, name=f"rbs{h}{qc}", tag="rbs")
                    srd = srow_d[idx : idx + 1, :]
                    nc.gpsimd.dma_start(
                        out=rb_sb,
                        in_=bass.AP(
                            tensor=srd.tensor,
                            offset=srd.offset,
                            ap=[[0, 128]] + srd.ap[1:],
                        ),
                    )
                    for half in range(2):
                        nc.vector.tensor_mul(
                            avTn_big[:, 2 * h + half, qs], av_ps[half], rb_sb
                        )

        # =========== Phase F: fc + residual + layernorm ===========
        with ExitStack() as ph:
            wp = ph.enter_context(tc.tile_pool(name="w_fc", bufs=1))
            wk = ph.enter_context(tc.tile_pool(name="ln_work", bufs=3))
            ps = ph.enter_context(tc.tile_pool(name="fc_ps", bufs=2, space="PSUM"))
            w_sb = wp.tile([128, DT, D], bf16, name="w_sb_fc")
            nc.sync.dma_start(w_sb, WfcT.rearrange("(t p) n -> p t n", p=128))

            sd = nc.vector.BN_STATS_DIM
            for q_t in range(LT):
                qsl = slice(q_t * 128, (q_t + 1) * 128)
                resid = wk.tile([128, D], f32, name=f"res{q_t}", tag="resid")
                nc.sync.dma_start(resid, q_nat[qsl, :])
                fc_ps = [
                    ps.tile([128, 512], f32, name=f"fc{q_t}_{i}", tag=f"fc{i}")
                    for i in range(2)
                ]
                for dc in range(2):
                    for din in range(DT):
                        nc.tensor.matmul(
                            fc_ps[dc],
                            avTn_big[:, din, q_t * 128 : (q_t + 1) * 128],
                            w_sb[:, din, dc * 512 : (dc + 1) * 512],
                            start=(din == 0),
                            stop=(din == DT - 1 and not use_bfc),
                        )
                    if use_bfc:
                        nc.tensor.matmul(
                            fc_ps[dc],
                            ones_row,
                            bfc_sb[:, dc * 512 : (dc + 1) * 512],
                            start=False,
                            stop=True,
                        )
                x = wk.tile([128, D], f32, name=f"x{q_t}", tag="x")
                for dc in range(2):
                    nc.vector.tensor_add(
                        x[:, dc * 512 : (dc + 1) * 512],
                        fc_ps[dc],
                        resid[:, dc * 512 : (dc + 1) * 512],
                    )
                stats = wk.tile([128, 2, sd], f32, name=f"stats{q_t}", tag="stats")
                nc.vector.bn_stats(stats[:, 0, :], x[:, 0:512])
                nc.vector.bn_stats(stats[:, 1, :], x[:, 512:1024])
                mv = wk.tile([128, nc.vector.BN_AGGR_DIM], f32, name=f"mv{q_t}", tag="mv")
                nc.vector.bn_aggr(mv, stats)
                rstd = wk.tile([128, 1], f32, name=f"rstd{q_t}", tag="rstd")
                nc.scalar.activation(rstd, mv[:, 1:2], AF.Sqrt, bias=eps_t)
                nc.vector.reciprocal(rstd, rstd)
                y = wk.tile([128, D], f32, name=f"y{q_t}", tag="y")
                nc.vector.tensor_scalar(
                    out=y,
                    in0=x,
                    scalar1=mv[:, 0:1],
                    scalar2=rstd,
                    op0=mybir.AluOpType.subtract,
                    op1=mybir.AluOpType.mult,
                )
                if use_gamma:
                    nc.vector.tensor_mul(y, y, gamma_bc)
                if use_beta:
                    nc.vector.tensor_add(y, y, beta_bc)
                nc.sync.dma_start(out[qsl, :], y)

    nc.compile()
    return nc


def TileCtx(nc):
    return tile.TileContext(nc)


_cache = {}


def _get_program(flags):
    key = tuple(sorted(flags.items()))
    if key not in _cache:
        _cache[key] = build(**flags)
    return _cache[key]


def kernel(**inputs):
    global LAST_RESULT
    q = np.asarray(inputs["q"], dtype=np.float32)
    k = np.asarray(inputs["k"], dtype=np.float32)
    v = np.asarray(inputs["v"], dtype=np.float32)
    mask = np.asarray(inputs["mask"])
    Wq = np.asarray(inputs["Wq"], dtype=np.float32)
    bq = np.asarray(inputs["bq"], dtype=np.float32)
    Wk = np.asarray(inputs["Wk"], dtype=np.float32)
    bk = np.asarray(inputs["bk"], dtype=np.float32)
    Wv = np.asarray(inputs["Wv"], dtype=np.float32)
    bv = np.asarray(inputs["bv"], dtype=np.float32)
    Wfc = np.asarray(inputs["Wfc"], dtype=np.float32)
    bfc = np.asarray(inputs["bfc"], dtype=np.float32)
    gamma = np.asarray(inputs["gamma"], dtype=np.float32)
    beta = np.asarray(inputs["beta"], dtype=np.float32)

    flags = dict(
        use_bqk=bool(np.any(bq) or np.any(bk)),
        use_bv=bool(np.any(bv)),
        use_bfc=bool(np.any(bfc)),
        use_gamma=bool(np.any(gamma != 1.0)),
        use_beta=bool(np.any(beta)),
    )
    nc = _get_program(flags)

    WqT = Wq.T.astype(ml_dtypes.bfloat16)
    WkT = Wk.T.astype(ml_dtypes.bfloat16)
    WvT = Wv.T.astype(ml_dtypes.bfloat16)
    WfcT = Wfc.T.astype(ml_dtypes.bfloat16)

    neg = np.array(MASK_NEG, dtype=ml_dtypes.bfloat16)
    zero = np.array(0.0, dtype=ml_dtypes.bfloat16)

    shared = dict(WqT=WqT, WkT=WkT, WvT=WvT, WfcT=WfcT)
    if flags["use_bqk"]:
        shared["bq_cols"] = np.ascontiguousarray(bq.reshape(DT, 128).T)
        shared["bk_cols"] = np.ascontiguousarray(bk.reshape(DT, 128).T)
    if flags["use_bv"]:
        shared["bv_row"] = bv.reshape(1, D)
    if flags["use_bfc"]:
        shared["bfc_row"] = bfc.reshape(1, D)
    if flags["use_gamma"]:
        shared["gamma_row"] = gamma.reshape(1, D)
    if flags["use_beta"]:
        shared["beta_row"] = beta.reshape(1, D)

    in_maps = []
    for b in range(B):
        m = dict(shared)
        m["qT"] = q[b].T.astype(ml_dtypes.bfloat16)
        m["kT"] = k[b].T.astype(ml_dtypes.bfloat16)
        m["vT"] = v[b].T.astype(ml_dtypes.bfloat16)
        m["q_nat"] = np.ascontiguousarray(q[b])
        m["maskbT"] = np.where(mask[b].T, neg, zero)
        in_maps.append(m)

    LAST_RESULT = run_bass_kernel_spmd(nc, in_maps, core_ids=list(range(B)))
    return np.stack([r["out"] for r in LAST_RESULT.results], axis=0)


# revision 10
# speedup vs baseline: 3.7431x; 1.1035x over previous
"""Trainium2 Bass kernel for a full MHA block (proj -> masked softmax attention
-> fc -> residual -> layernorm), data-parallel over batch across 8 NeuronCores.

Layout strategy (per core, one batch element):
  - Host pre-transposes weights (W.T) and activations (q.T/k.T/v.T) so every
    matmul contraction dim lands on SBUF partitions with zero on-chip
    transposes.
  - Scores are computed *transposed* (S.T[lk, q]) so that the attention
    probabilities are directly usable as the moving operand of the attn@v
    matmul (contraction over lk = partitions).
  - Softmax without max-subtraction: raw scores are bounded (|S|/16 < ~10),
    masked entries get -2^30 added pre-exp so exp underflows to exactly 0.
    Fully-masked rows then produce sum==0 -> recip(sum+1e-30) finite -> attn
    row exactly 0, matching the reference's NaN-fix.
  - Sum-of-exp over partitions via a ones-vector matmul on the PE; the
    reciprocal row is broadcast back to 128 partitions with a K=1 matmul.
"""

import sys

if "/opt/trn_rl_repo" not in sys.path:
    sys.path.insert(0, "/opt/trn_rl_repo")

from contextlib import ExitStack

import ml_dtypes
import numpy as np

import concourse.bass as bass
import concourse.tile as tile
from concourse import bacc, mybir
from concourse.bass_utils import run_bass_kernel_spmd

B, L, D, H = 8, 2048, 1024, 4
DK = D // H  # 256
LT = L // 128  # 16 l-tiles of 128
DT = D // 128  # 8 d-tiles of 128
NQ = 4  # attention q chunks
QW = L // NQ  # 512 q columns per chunk
EPS = 1e-5
INV_TEMP = 1.0 / 16.0  # 1/sqrt(DK)
MASK_NEG = -float(2**30)

f32 = mybir.dt.float32
bf16 = mybir.dt.bfloat16

AF = mybir.ActivationFunctionType

# set by kernel() after each run; test.py reads it for the HW time
LAST_RESULT = None


def build(use_bqk, use_bv, use_bfc, use_gamma, use_beta):
    nc = bacc.Bacc("TRN2", target_bir_lowering=False, debug=False, num_devices=B)

    # ---- per-core I/O (full tensors for one batch element) ----
    qT = nc.dram_tensor("qT", [D, L], bf16, kind="ExternalInput")
    kT = nc.dram_tensor("kT", [D, L], bf16, kind="ExternalInput")
    vT = nc.dram_tensor("vT", [D, L], bf16, kind="ExternalInput")
    q_nat = nc.dram_tensor("q_nat", [L, D], f32, kind="ExternalInput")
    maskbT = nc.dram_tensor("maskbT", [L, L], bf16, kind="ExternalInput")
    WqT = nc.dram_tensor("WqT", [D, D], bf16, kind="ExternalInput")
    WkT = nc.dram_tensor("WkT", [D, D], bf16, kind="ExternalInput")
    WvT = nc.dram_tensor("WvT", [D, D], bf16, kind="ExternalInput")
    WfcT = nc.dram_tensor("WfcT", [D, D], bf16, kind="ExternalInput")
    if use_bqk:
        # bias for dout tile j lives in column j as a [128,1] per-partition vec
        bq_cols = nc.dram_tensor("bq_cols", [128, DT], f32, kind="ExternalInput")
        bk_cols = nc.dram_tensor("bk_cols", [128, DT], f32, kind="ExternalInput")
    if use_bv:
        bv_row = nc.dram_tensor("bv_row", [1, D], f32, kind="ExternalInput")
    if use_bfc:
        bfc_row = nc.dram_tensor("bfc_row", [1, D], f32, kind="ExternalInput")
    if use_gamma:
        gamma_row = nc.dram_tensor("gamma_row", [1, D], f32, kind="ExternalInput")
    if use_beta:
        beta_row = nc.dram_tensor("beta_row", [1, D], f32, kind="ExternalInput")
    out = nc.dram_tensor("out", [L, D], f32, kind="ExternalOutput")

    with TileCtx(nc) as tc, ExitStack() as top:
        dram = top.enter_context(tc.tile_pool(name="dram", bufs=1, space="DRAM"))
        qpT = dram.tile([D, L], bf16, name="qpT")  # [dout, l]
        kpT = dram.tile([D, L], bf16, name="kpT")  # [dout, l]
        srow_d = dram.tile([H * NQ, QW], f32, name="srow_d")  # recip bounce rows

        bigs = top.enter_context(tc.tile_pool(name="bigs", bufs=1))
        vp_big = bigs.tile([128, LT, D], bf16, name="vp_big")  # [l-part, lt, dout]
        avTn_big = bigs.tile([128, DT, L], bf16, name="avTn_big")  # [dchunk, t, q]
        maskp = top.enter_context(tc.tile_pool(name="maskp", bufs=1))
        mask_sb = maskp.tile([128, LT, L], bf16, name="mask_sb")

        singles = top.enter_context(tc.tile_pool(name="singles", bufs=1))
        ones_col = singles.tile([128, 1], bf16, name="ones_col")
        nc.vector.memset(ones_col, 1.0)
        ones_row = singles.tile([1, 128], f32, name="ones_row")
        nc.vector.memset(ones_row, 1.0)
        eps_t = singles.tile([128, 1], f32, name="eps_t")
        nc.vector.memset(eps_t, EPS)
        ident = singles.tile([128, 128], bf16, name="ident")
        nc.gpsimd.memset(ident, 0.0)
        nc.gpsimd.affine_select(
            out=ident,
            in_=ident,
            compare_op=mybir.AluOpType.not_equal,
            fill=1.0,
            base=0,
            pattern=[[-1, 128]],
            channel_multiplier=1,
        )
        if use_bqk:
            bq_sb = singles.tile([128, DT], f32, name="bq_sb")
            nc.sync.dma_start(bq_sb, bq_cols[:, :])
            bk_sb = singles.tile([128, DT], f32, name="bk_sb")
            nc.sync.dma_start(bk_sb, bk_cols[:, :])
        if use_bv:
            bv_sb = singles.tile([1, D], f32, name="bv_sb")
            nc.sync.dma_start(bv_sb, bv_row[:, :])
        if use_bfc:
            bfc_sb = singles.tile([1, D], f32, name="bfc_sb")
            nc.sync.dma_start(bfc_sb, bfc_row[:, :])
        if use_gamma:
            gamma_bc = singles.tile([128, D], f32, name="gamma_bc")
            g = gamma_row[:, :]
            nc.gpsimd.dma_start(
                out=gamma_bc,
                in_=bass.AP(tensor=g.tensor, offset=g.offset, ap=[[0, 128], [1, D]]),
            )
        if use_beta:
            beta_bc = singles.tile([128, D], f32, name="beta_bc")
            bt = beta_row[:, :]
            nc.gpsimd.dma_start(
                out=beta_bc,
                in_=bass.AP(tensor=bt.tensor, offset=bt.offset, ap=[[0, 128], [1, D]]),
            )

        # =========== Phase P: projections ===========
        # q/k: out_pT[dout, l] = W @ x.T  (lhsT = W.T[din,dout], rhs = x.T[din,l])
        # x streamed in l-chunks; w double-buffered so q/k/v overlap.
        mask_r = maskbT.rearrange("(t p) q -> p t q", p=128)
        with ExitStack() as ph:
            wp = ph.enter_context(tc.tile_pool(name="w_pool", bufs=2))
            xp = ph.enter_context(tc.tile_pool(name="x_pool", bufs=3))
            ev = ph.enter_context(tc.tile_pool(name="ev_pool", bufs=3))
            ps = ph.enter_context(tc.tile_pool(name="ps_proj", bufs=1, space="PSUM"))

            first = True
            for name, WT_d, xT_d, outT, bias_sb in (
                ("q", WqT, qT, qpT, bq_sb if use_bqk else None),
                ("k", WkT, kT, kpT, bk_sb if use_bqk else None),
            ):
                w_sb = wp.tile([128, DT, D], bf16, name=f"w_{name}", tag="w")
                nc.sync.dma_start(w_sb, WT_d.rearrange("(t p) n -> p t n", p=128))
                xr = xT_d.rearrange("(t p) l -> p t l", p=128)
                for qc in range(NQ):
                    x_c = xp.tile([128, DT, QW], bf16, name=f"x_{name}{qc}", tag="xc")
                    nc.sync.dma_start(x_c, xr[:, :, qc * QW : (qc + 1) * QW])
                    if first:
                        # prefetch mask behind the first tensor's activations
                        nc.sync.dma_start(
                            mask_sb[:, qc * 4 : (qc + 1) * 4, :],
                            mask_r[:, qc * 4 : (qc + 1) * 4, :],
                        )
                    for dout_t in range(DT):
                        pst = ps.tile(
                            [128, QW], f32, name=f"pp{name}{qc}{dout_t}", tag="pp", bufs=3
                        )
                        for din in range(DT):
                            nc.tensor.matmul(
                                pst,
                                w_sb[:, din, dout_t * 128 : (dout_t + 1) * 128],
                                x_c[:, din, :],
                                start=(din == 0),
                                stop=(din == DT - 1),
                            )
                        evt = ev.tile(
                            [128, QW], bf16, name=f"ev{name}{qc}{dout_t}", tag="evt"
                        )
                        if bias_sb is not None:
                            nc.scalar.activation(
                                evt,
                                pst,
                                AF.Identity,
                                bias=bias_sb[:, dout_t : dout_t + 1],
                            )
                        else:
                            nc.scalar.copy(evt, pst)
                        nc.sync.dma_start(
                            outT[
                                dout_t * 128 : (dout_t + 1) * 128,
                                qc * QW : (qc + 1) * QW,
                            ],
                            evt,
                        )
                first = False

            # v: vp[l, dout] = (v.T).T @ W.T  (lhsT = v.T[din, l], rhs = W.T)
            w_sb = wp.tile([128, DT, D], bf16, name="w_v", tag="w")
            nc.sync.dma_start(w_sb, WvT.rearrange("(t p) n -> p t n", p=128))
            vr = vT.rearrange("(t p) l -> p t l", p=128)
            for g in range(NQ):
                x_c = xp.tile([128, DT, QW], bf16, name=f"x_v{g}", tag="xc")
                nc.sync.dma_start(x_c, vr[:, :, g * QW : (g + 1) * QW])
                for j in range(4):
                    l_t = g * 4 + j
                    pst2 = [
                        ps.tile(
                            [128, 512], f32, name=f"ppv{l_t}_{i}", tag=f"ppv{i}", bufs=2
                        )
                        for i in range(2)
                    ]
                    for din in range(DT):
                        lhsT = x_c[:, din, j * 128 : (j + 1) * 128]
                        for dc in range(2):
                            nc.tensor.matmul(
                                pst2[dc],
                                lhsT,
                                w_sb[:, din, dc * 512 : (dc + 1) * 512],
                                start=(din == 0),
                                stop=(din == DT - 1 and not use_bv),
                            )
                    if use_bv:
                        for dc in range(2):
                            nc.tensor.matmul(
                                pst2[dc],
                                ones_row,
                                bv_sb[:, dc * 512 : (dc + 1) * 512],
                                start=False,
                                stop=True,
                            )
                    for dc in range(2):
                        nc.scalar.copy(
                            vp_big[:, l_t, dc * 512 : (dc + 1) * 512], pst2[dc]
                        )

        # =========== Phase A: attention per head ===========
        with ExitStack() as ph:
            hp = ph.enter_context(tc.tile_pool(name="headp", bufs=2))
            wk = ph.enter_context(tc.tile_pool(name="attn_work", bufs=3))
            ps = ph.enter_context(tc.tile_pool(name="attn_ps", bufs=1, space="PSUM"))

            for h in range(H):
                r0 = h * DK
                kp_sb = hp.tile([128, 2, L], bf16, name=f"kp_sb{h}", tag="kp")
                nc.sync.dma_start(
                    kp_sb, kpT[r0 : r0 + DK, :].rearrange("(c p) q -> p c q", p=128)
                )
                qp_sb = hp.tile([128, 2, L], bf16, name=f"qp_sb{h}", tag="qp")
                nc.sync.dma_start(
                    qp_sb, qpT[r0 : r0 + DK, :].rearrange("(c p) q -> p c q", p=128)
                )
                for qc in range(NQ):
                    qs = slice(qc * QW, (qc + 1) * QW)
                    av_ps = [
                        ps.tile([128, QW], f32, name=f"av{h}_{qc}_{i}", tag=f"av{i}", bufs=2)
                        for i in range(2)
                    ]
                    sum_ps = ps.tile([1, QW], f32, name=f"sum{h}_{qc}", tag="sum")
                    for lk in range(LT):
                        st_ps = ps.tile(
                            [128, QW], f32, name=f"st{h}_{qc}_{lk}", tag="st", bufs=3
                        )
                        nc.tensor.matmul(
                            st_ps,
                            kp_sb[:, 0, lk * 128 : (lk + 1) * 128],
                            qp_sb[:, 0, qs],
                            start=True,
                            stop=False,
                        )
                        nc.tensor.matmul(
                            st_ps,
                            kp_sb[:, 1, lk * 128 : (lk + 1) * 128],
                            qp_sb[:, 1, qs],
                            start=False,
                            stop=True,
                        )
                        stm = wk.tile(
                            [128, QW], f32, name=f"stm{h}{qc}{lk}", tag="stm", bufs=4
                        )
                        nc.vector.tensor_add(stm, st_ps, mask_sb[:, lk, qs])
                        pt = wk.tile([128, QW], bf16, name=f"pt{h}{qc}{lk}", tag="pt", bufs=6)
                        nc.scalar.activation(pt, stm, AF.Exp, scale=INV_TEMP)
                        nc.tensor.matmul(
                            av_ps[0],
                            vp_big[:, lk, r0 : r0 + 128],
                            pt,
                            start=(lk == 0),
                            stop=(lk == LT - 1),
                        )
                        nc.tensor.matmul(
                            av_ps[1],
                            vp_big[:, lk, r0 + 128 : r0 + 256],
                            pt,
                            start=(lk == 0),
                            stop=(lk == LT - 1),
                        )
                        nc.tensor.matmul(
                            sum_ps,
                            ones_col,
                            pt,
                            start=(lk == 0),
                            stop=(lk == LT - 1),
                        )
                    srow = wk.tile([1, QW], f32, name=f"srow{h}{qc}", tag="srow")
                    nc.vector.tensor_scalar_add(srow, sum_ps, 1e-30)
                    nc.vector.reciprocal_approx_fast(srow, srow)
                    idx = h * NQ + qc
                    nc.sync.dma_start(srow_d[idx : idx + 1, :], srow)
                    rb_sb = wk.tile([128, QW], f32, name=f"rbs{h}{qc}", tag="rbs")
                    srd = srow_d[idx : idx + 1, :]
                    nc.gpsimd.dma_start(
                        out=rb_sb,
                        in_=bass.AP(
                            tensor=srd.tensor,
                            offset=srd.offset,
                            ap=[[0, 128]] + srd.ap[1:],
                        ),
                    )
                    for half in range(2):
                        nc.vector.tensor_mul(
                            avTn_big[:, 2 * h + half, qs], av_ps[half], rb_sb
                        )

        # =========== Phase F: fc + residual + layernorm ===========
        with ExitStack() as ph:
            wp = ph.enter_context(tc.tile_pool(name="w_fc", bufs=1))
            wk = ph.enter_context(tc.tile_pool(name="ln_work", bufs=3))
            ps = ph.enter_context(tc.tile_pool(name="fc_ps", bufs=2, space="PSUM"))
            w_sb = wp.tile([128, DT, D], bf16, name="w_sb_fc")
            nc.sync.dma_start(w_sb, WfcT.rearrange("(t p) n -> p t n", p=128))

            sd = nc.vector.BN_STATS_DIM
            for q_t in range(LT):
                qsl = slice(q_t * 128, (q_t + 1) * 128)
                resid = wk.tile([128, D], f32, name=f"res{q_t}", tag="resid")
                nc.sync.dma_start(resid, q_nat[qsl, :])
                fc_ps = [
                    ps.tile([128, 512], f32, name=f"fc{q_t}_{i}", tag=f"fc{i}")
                    for i in range(2)
                ]
                for dc in range(2):
                    for din in range(DT):
                        nc.tensor.matmul(
                            fc_ps[dc],
                            avTn_big[:, din, q_t * 128 : (q_t + 1) * 128],
                            w_sb[:, din, dc * 512 : (dc + 1) * 512],
                            start=(din == 0),
                            stop=(din == DT - 1 and not use_bfc),
                        )
                    if use_bfc:
                        nc.tensor.matmul(
                            fc_ps[dc],
                            ones_row,
                            bfc_sb[:, dc * 512 : (dc + 1) * 512],
                            start=False,
                            stop=True,
                        )
                x = wk.tile([128, D], f32, name=f"x{q_t}", tag="x")
                for dc in range(2):
                    nc.vector.tensor_add(
                        x[:, dc * 512 : (dc + 1) * 512],
                        fc_ps[dc],
                        resid[:, dc * 512 : (dc + 1) * 512],
                    )
                stats = wk.tile([128, 2, sd], f32, name=f"stats{q_t}", tag="stats")
                nc.vector.bn_stats(stats[:, 0, :], x[:, 0:512])
                nc.vector.bn_stats(stats[:, 1, :], x[:, 512:1024])
                mv = wk.tile([128, nc.vector.BN_AGGR_DIM], f32, name=f"mv{q_t}", tag="mv")
                nc.vector.bn_aggr(mv, stats)
                rstd = wk.tile([128, 1], f32, name=f"rstd{q_t}", tag="rstd")
                nc.scalar.activation(rstd, mv[:, 1:2], AF.Sqrt, bias=eps_t)
                nc.vector.reciprocal(rstd, rstd)
                y = wk.tile([128, D], f32, name=f"y{q_t}", tag="y")
                nc.vector.tensor_scalar(
                    out=y,
                    in0=x,
                    scalar1=mv[:, 0:1],
                    scalar2=rstd,
                    op0=mybir.AluOpType.subtract,
                    op1=mybir.AluOpType.mult,
                )
                if use_gamma:
                    nc.vector.tensor_mul(y, y, gamma_bc)
                if use_beta:
                    nc.vector.tensor_add(y, y, beta_bc)
                nc.sync.dma_start(out[qsl, :], y)

    nc.compile()
    return nc


def TileCtx(nc):
    return tile.TileContext(nc)


_cache = {}


def _get_program(flags):
    key = tuple(sorted(flags.items()))
    if key not in _cache:
        _cache[key] = build(**flags)
    return _cache[key]


def kernel(**inputs):
    global LAST_RESULT
    q = np.asarray(inputs["q"], dtype=np.float32)
    k = np.asarray(inputs["k"], dtype=np.float32)
    v = np.asarray(inputs["v"], dtype=np.float32)
    mask = np.asarray(inputs["mask"])
    Wq = np.asarray(inputs["Wq"], dtype=np.float32)
    bq = np.asarray(inputs["bq"], dtype=np.float32)
    Wk = np.asarray(inputs["Wk"], dtype=np.float32)
    bk = np.asarray(inputs["bk"], dtype=np.float32)
    Wv = np.asarray(inputs["Wv"], dtype=np.float32)
    bv = np.asarray(inputs["bv"], dtype=np.float32)
    Wfc = np.asarray(inputs["Wfc"], dtype=np.float32)
    bfc = np.asarray(inputs["bfc"], dtype=np.float32)
    gamma = np.asarray(inputs["gamma"], dtype=np.float32)
    beta = np.asarray(inputs["beta"], dtype=np.float32)

    flags = dict(
        use_bqk=bool(np.any(bq) or np.any(bk)),
        use_bv=bool(np.any(bv)),
        use_bfc=bool(np.any(bfc)),
        use_gamma=bool(np.any(gamma != 1.0)),
        use_beta=bool(np.any(beta)),
    )
    nc = _get_program(flags)

    WqT = Wq.T.astype(ml_dtypes.bfloat16)
    WkT = Wk.T.astype(ml_dtypes.bfloat16)
    WvT = Wv.T.astype(ml_dtypes.bfloat16)
    WfcT = Wfc.T.astype(ml_dtypes.bfloat16)

    neg = np.array(MASK_NEG, dtype=ml_dtypes.bfloat16)
    zero = np.array(0.0, dtype=ml_dtypes.bfloat16)

    shared = dict(WqT=WqT, WkT=WkT, WvT=WvT, WfcT=WfcT)
    if flags["use_bqk"]:
        shared["bq_cols"] = np.ascontiguousarray(bq.reshape(DT, 128).T)
        shared["bk_cols"] = np.ascontiguousarray(bk.reshape(DT, 128).T)
    if flags["use_bv"]:
        shared["bv_row"] = bv.reshape(1, D)
    if flags["use_bfc"]:
        shared["bfc_row"] = bfc.reshape(1, D)
    if flags["use_gamma"]:
        shared["gamma_row"] = gamma.reshape(1, D)
    if flags["use_beta"]:
        shared["beta_row"] = beta.reshape(1, D)

    in_maps = []
    for b in range(B):
        m = dict(shared)
        m["qT"] = q[b].T.astype(ml_dtypes.bfloat16)
        m["kT"] = k[b].T.astype(ml_dtypes.bfloat16)
        m["vT"] = v[b].T.astype(ml_dtypes.bfloat16)
        m["q_nat"] = np.ascontiguousarray(q[b])
        m["maskbT"] = np.where(mask[b].T, neg, zero)
        in_maps.append(m)

    LAST_RESULT = run_bass_kernel_spmd(nc, in_maps, core_ids=list(range(B)))
    return np.stack([r["out"] for r in LAST_RESULT.results], axis=0)
